# revision 18
# baseline (speedup 1.0000x reference)
"""AdderNet 2D convolution on 8 TRN2 NeuronCores.

out[n,co,h,w] = -sum_{ci,kh,kw} |xpad[n,ci,h+kh,w+kw] - w[co,ci,kh,kw]|

Sharding: data-parallel over the batch dim (16 images -> 2 per core),
weight replicated.  No collectives needed (forward pass only).

Math: |x - w| = x + w - 2*min(x, w), so

  -sum |x - w| = 2*sum min(x, w) - sum x - sum w

The heavy term is one single-op DVE tensor_scalar per (co, tap):
y = min(xpad, w[co,ci,kh,kw]) in bf16 (4x perf mode), evaluated over the
full padded plane so every instruction is contiguous/aligned.  TensorE
reduces partitions with a constant +2 block stationary into PSUM
(accumulating the 9 taps); the (kh,kw) tap shift is applied by the
matmul's strided moving-view.  "sum x" is accumulated by 252 extra
matmuls with an all-(-1) stationary; "sum w" comes in as a tiny
host-precomputed per-partition bias, applied in the epilogue.

Per-core layout:
  - 128 SBUF partitions = img*64 + ci  (2 images per core)
  - psum/output partition p = 32*(co//16) + 2*(co%16) + img
    (TensorE column-tiling: 4 strips of 32, one per co-group)
  - zero padding in xpad contributes min(0, w) terms and the matching
    zeros in sum x, exactly reproducing the reference's |0 - w| border
    terms.
"""

import numpy as np

try:
    from concourse import bacc, mybir, tile
except ImportError:  # pragma: no cover - fallback when sitecustomize absent
    import sys

    sys.path.insert(0, "/opt/trn_rl_repo")
    from concourse import bacc, mybir, tile

from concourse.bass_utils import run_bass_kernel_spmd

N, C, H, W = 16, 64, 56, 56
CO, K = 64, 3
NCORES = 8
NLOC = N // NCORES  # images per core = 2
HP = H + 2  # padded plane height
WP = W + 2
L = H * W  # 3136 output pixels
CHUNK_ROWS = 8  # output rows per psum bank chunk
NCHUNK = H // CHUNK_ROWS  # 7
CHUNK = CHUNK_ROWS * W  # 448 <= 512 fp32 / psum bank

_nc_cache = None


def build_nc():
    nc = bacc.Bacc(
        "TRN2",
        target_bir_lowering=False,
        debug=False,
        num_devices=NCORES,
    )
    f32 = mybir.dt.float32
    bf16 = mybir.dt.bfloat16

    x_d = nc.dram_tensor("x", [NLOC, C, H, W], f32, kind="ExternalInput")
    w_d = nc.dram_tensor("w", [CO, C, K, K], f32, kind="ExternalInput")
    # swn[p, 0] = -sum_{ci,kh,kw} w[co(p)] at psum partition p (host-computed)
    swn_d = nc.dram_tensor("swn", [128, 1], f32, kind="ExternalInput")
    # out rows are psum-partition-major: p = 32*(co//16) + 2*(co%16) + img;
    # the host-side gather untangles this ordering (cheap numpy transpose).
    o_d = nc.dram_tensor("out", [128, L], f32, kind="ExternalOutput")

    with tile.TileContext(nc) as tc:
        with (
            tc.tile_pool(name="const", bufs=1) as cpool,
            tc.tile_pool(name="ypool", bufs=4) as ypool,
            tc.tile_pool(name="psum", bufs=1, space="PSUM") as ppool,
        ):
            xstage = cpool.tile([128, H, W], f32)
            xpad = cpool.tile([128, HP, WP], bf16)
            wbias = cpool.tile([128, CO, K * K], f32)
            wneg = cpool.tile([128, CO, K * K], f32)
            swn = cpool.tile([128, 1], f32)
            # stat2[:, c, :]: [128, 32] stationary, col 2c+i = +2 on the
            # img-i partition half, else 0  (the 2*min reduction).
            stat2 = cpool.tile([128, 16, 32], bf16)
            # statm[:, c, :]: same pattern with -1 (the -|x-w| ACT tiles).
            statm = cpool.tile([128, 16, 32], bf16)
            # statn: [128, 32] all-columns -1 on matching img half (sum-x).
            statn = cpool.tile([128, 32], bf16)
            out_sb = cpool.tile([128, L], f32)

            # ---- loads -------------------------------------------------
            nc.sync.dma_start(xstage[:], x_d.ap().rearrange("n c h w -> (n c) h w"))
            # wbias[p = img*64 + ci, co, kh*3 + kw] = w[co, ci, kh, kw]
            wv = w_d.ap().rearrange("co ci kh kw -> ci co (kh kw)")
            nc.sync.dma_start(wbias[0:64], wv)
            nc.sync.dma_start(wbias[64:128], wv)
            nc.sync.dma_start(swn[:], swn_d.ap())

            # ---- constants --------------------------------------------
            nc.vector.memset(stat2[:], 0.0)
            nc.vector.memset(statm[:], 0.0)
            for c in range(16):
                nc.vector.memset(stat2[0:64, c, 2 * c : 2 * c + 1], 2.0)
                nc.vector.memset(stat2[64:128, c, 2 * c + 1 : 2 * c + 2], 2.0)
                nc.vector.memset(statm[0:64, c, 2 * c : 2 * c + 1], -1.0)
                nc.vector.memset(statm[64:128, c, 2 * c + 1 : 2 * c + 2], -1.0)
            nc.vector.memset(statn[:], 0.0)
            nc.vector.memset(statn[0:64, 0:32:2], -1.0)
            nc.vector.memset(statn[64:128, 1:32:2], -1.0)
            nc.vector.tensor_scalar(
                wneg[:], wbias[:], -1.0, None, op0=mybir.AluOpType.mult
            )

            # ---- pad + cast to bf16 -----------------------------------
            nc.vector.memset(xpad[:], 0.0)
            nc.vector.tensor_copy(xpad[:, 1 : H + 1, 1 : W + 1], xstage[:])

            psums = [
                ppool.tile([128, CHUNK], f32, name=f"ps{f}", tag=f"ps{f}")
                for f in range(NCHUNK)
            ]

            taps = [(kh, kw) for kh in range(K) for kw in range(K)]
            NMIN = 7  # taps[0:NMIN] via DVE min-trick, the rest via ACT abs
            xflat = xpad[:].rearrange("p h w -> p (h w)")
            YF = W * WP  # 3248: rows kh..kh+55 of the padded plane, flat

            # ---- sum-x accumulation (also the PE warm-up burst) --------
            for f in range(NCHUNK):
                r0 = f * CHUNK_ROWS
                for t in range(NMIN):
                    kh, kw = taps[t]
                    rhs = xpad[:, r0 + kh : r0 + kh + CHUNK_ROWS, kw : kw + W]
                    for g in range(4):
                        nc.tensor.matmul(
                            psums[f][32 * g : 32 * g + 32, :],
                            statn[:],
                            rhs,
                            start=(t == 0),
                            stop=False,
                            tile_position=(0, 32 * g),
                        )

            # ---- main loop -------------------------------------------
            # DVE tiles: y = min(x, w)        -> stationary +2
            # ACT tiles: y = |x - w|          -> stationary -1
            # Both computed flat over padded rows kh..kh+55 (contiguous,
            # aligned); the matmul moving-view applies the (kh, kw) shift.
            for cc in range(16):
                for t, (kh, kw) in enumerate(taps):
                    for g in range(4):
                        co = 16 * g + cc
                        y = ypool.tile([128, YF], bf16, tag="y")
                        src = xflat[:, kh * WP : kh * WP + YF]
                        if t < NMIN:
                            nc.vector.tensor_scalar(
                                y[:],
                                src,
                                wbias[:, co, t : t + 1],
                                None,
                                op0=mybir.AluOpType.min,
                            )
                            lhsT = stat2[:, cc, :]
                        else:
                            nc.scalar.activation(
                                y[:],
                                src,
                                mybir.ActivationFunctionType.Abs,
                                bias=wneg[:, co, t : t + 1],
                            )
                            lhsT = statm[:, cc, :]
                        y3 = y[:].rearrange("p (r c) -> p r c", c=WP)
                        for f in range(NCHUNK):
                            r0 = f * CHUNK_ROWS
                            rhs = y3[:, r0 : r0 + CHUNK_ROWS, kw : kw + W]
                            nc.tensor.matmul(
                                psums[f][32 * g : 32 * g + 32, :],
                                lhsT,
                                rhs,
                                start=False,
                                stop=(cc == 15 and t == len(taps) - 1),
                                tile_position=(0, 32 * g),
                            )

            # ---- epilogue: out = psum + (-sum w), psum -> sbuf -> dram --
            for f in range(NCHUNK):
                nc.scalar.activation(
                    out_sb[:, f * CHUNK : (f + 1) * CHUNK],
                    psums[f][:],
                    mybir.ActivationFunctionType.Identity,
                    bias=swn[:],
                )
            nc.sync.dma_start(o_d.ap(), out_sb[:])

    nc.compile()
    return nc


def get_nc():
    global _nc_cache
    if _nc_cache is None:
        _nc_cache = build_nc()
    return _nc_cache


def make_in_maps(x, w):
    x = np.ascontiguousarray(x, dtype=np.float32)
    w = np.ascontiguousarray(w, dtype=np.float32)
    # -sum w[co] over the min-trick taps (taps[0:7] in kh*3+kw order),
    # scattered to psum partitions p = 32*(co//16)+2*(co%16)+img
    swc = -w.reshape(CO, C, K * K)[:, :, :7].sum(axis=(1, 2))
    swn = np.empty((128, 1), dtype=np.float32)
    for co in range(CO):
        p = 32 * (co // 16) + 2 * (co % 16)
        swn[p, 0] = swc[co]
        swn[p + 1, 0] = swc[co]
    return [
        {"x": x[i * NLOC : (i + 1) * NLOC], "w": w, "swn": swn}
        for i in range(NCORES)
    ]


def unscramble(core_out):
    """[128, L] with row p = 32*(co//16) + 2*(co%16) + img -> [2, 64, 56, 56]."""
    return (
        core_out.reshape(4, 16, NLOC, H, W)
        .transpose(2, 0, 1, 3, 4)
        .reshape(NLOC, CO, H, W)
    )


def kernel(x, w):
    nc = get_nc()
    res = run_bass_kernel_spmd(nc, make_in_maps(x, w), core_ids=list(range(NCORES)))
    out = np.concatenate([unscramble(r["out"]) for r in res.results], axis=0)
    return np.ascontiguousarray(out, dtype=np.float32)


if __name__ == "__main__":
    x = np.random.randn(N, C, H, W).astype(np.float32)
    w = np.random.randn(CO, C, K, K).astype(np.float32)
    o = kernel(x, w)
    print("out", o.shape, o.dtype, float(o.mean()))


# revision 20
# speedup vs baseline: 1.4553x; 1.4553x over previous
"""AdderNet 2D convolution on 8 TRN2 NeuronCores.

out[n,co,h,w] = -sum_{ci,kh,kw} |xpad[n,ci,h+kh,w+kw] - w[co,ci,kh,kw]|

Sharding: data-parallel over the batch dim (16 images -> 2 per core),
weight replicated.  No collectives needed (forward pass only).

Math: |x - w| = x + w - 2*min(x, w), so

  -sum |x - w| = 2*sum min(x, w) - sum x - sum w

The heavy term is one single-op DVE tensor_scalar per (co, tap):
y = min(xpad, w[co,ci,kh,kw]) in bf16 (4x perf mode), evaluated over the
full padded plane so every instruction is contiguous/aligned.  TensorE
reduces partitions with a constant +2 block stationary into PSUM
(accumulating the 9 taps); the (kh,kw) tap shift is applied by the
matmul's strided moving-view.  "sum x" is accumulated by 252 extra
matmuls with an all-(-1) stationary; "sum w" comes in as a tiny
host-precomputed per-partition bias, applied in the epilogue.

Per-core layout:
  - 128 SBUF partitions = img*64 + ci  (2 images per core)
  - psum/output partition p = 32*(co//16) + 2*(co%16) + img
    (TensorE column-tiling: 4 strips of 32, one per co-group)
  - zero padding in xpad contributes min(0, w) terms and the matching
    zeros in sum x, exactly reproducing the reference's |0 - w| border
    terms.
"""

import numpy as np

try:
    from concourse import bacc, mybir, tile
except ImportError:  # pragma: no cover - fallback when sitecustomize absent
    import sys

    sys.path.insert(0, "/opt/trn_rl_repo")
    from concourse import bacc, mybir, tile

from concourse.bass_utils import run_bass_kernel_spmd

N, C, H, W = 16, 64, 56, 56
CO, K = 64, 3
NCORES = 8
NLOC = N // NCORES  # images per core = 2
HP = H + 2  # padded plane height
WP = W + 2
L = H * W  # 3136 output pixels
CHUNK_ROWS = 8  # output rows per psum bank chunk
NCHUNK = H // CHUNK_ROWS  # 7
CHUNK = CHUNK_ROWS * W  # 448 <= 512 fp32 / psum bank

_nc_cache = None


def build_nc():
    nc = bacc.Bacc(
        "TRN2",
        target_bir_lowering=False,
        debug=False,
        num_devices=NCORES,
    )
    f32 = mybir.dt.float32
    bf16 = mybir.dt.bfloat16

    x_d = nc.dram_tensor("x", [NLOC, C, H, W], f32, kind="ExternalInput")
    w_d = nc.dram_tensor("w", [CO, C, K, K], f32, kind="ExternalInput")
    # swn[p, 0] = -sum_{ci,kh,kw} w[co(p)] at psum partition p (host-computed)
    swn_d = nc.dram_tensor("swn", [128, 1], f32, kind="ExternalInput")
    # out rows are psum-partition-major: p = 32*(co//16) + 2*(co%16) + img;
    # the host-side gather untangles this ordering (cheap numpy transpose).
    o_d = nc.dram_tensor("out", [128, L], f32, kind="ExternalOutput")

    with tile.TileContext(nc) as tc:
        with (
            tc.tile_pool(name="const", bufs=1) as cpool,
            tc.tile_pool(name="ypool", bufs=6) as ypool,
            tc.tile_pool(name="apool", bufs=6) as apool,
            tc.tile_pool(name="psum", bufs=1, space="PSUM") as ppool,
        ):
            xstage = cpool.tile([128, H, W], f32)
            xpad = cpool.tile([128, HP, WP], bf16)
            wbias = cpool.tile([128, CO, K * K], f32)
            wneg = cpool.tile([128, CO, K * K], f32)
            swn = cpool.tile([128, 1], f32)
            # stat2[:, c, :]: [128, 32] stationary, col 2c+i = +2 on the
            # img-i partition half, else 0  (the 2*min reduction).
            stat2 = cpool.tile([128, 16, 32], bf16)
            # statm[:, c, :]: same pattern with -1 (the -|x-w| ACT tiles).
            statm = cpool.tile([128, 16, 32], bf16)
            # statn: [128, 32] all-columns -1 on matching img half (sum-x).
            statn = cpool.tile([128, 32], bf16)
            out_sb = cpool.tile([128, L], f32)

            # ---- loads -------------------------------------------------
            nc.sync.dma_start(xstage[:], x_d.ap().rearrange("n c h w -> (n c) h w"))
            # wbias[p = img*64 + ci, co, kh*3 + kw] = w[co, ci, kh, kw]
            wv = w_d.ap().rearrange("co ci kh kw -> ci co (kh kw)")
            nc.sync.dma_start(wbias[0:64], wv)
            nc.sync.dma_start(wbias[64:128], wv)
            nc.sync.dma_start(swn[:], swn_d.ap())

            # ---- constants --------------------------------------------
            nc.vector.memset(stat2[:], 0.0)
            nc.vector.memset(statm[:], 0.0)
            for c in range(16):
                nc.vector.memset(stat2[0:64, c, 2 * c : 2 * c + 1], 2.0)
                nc.vector.memset(stat2[64:128, c, 2 * c + 1 : 2 * c + 2], 2.0)
                nc.vector.memset(statm[0:64, c, 2 * c : 2 * c + 1], -1.0)
                nc.vector.memset(statm[64:128, c, 2 * c + 1 : 2 * c + 2], -1.0)
            nc.vector.memset(statn[:], 0.0)
            nc.vector.memset(statn[0:64, 0:32:2], -1.0)
            nc.vector.memset(statn[64:128, 1:32:2], -1.0)
            nc.vector.tensor_scalar(
                wneg[:], wbias[:], -1.0, None, op0=mybir.AluOpType.mult
            )

            # ---- pad + cast to bf16 -----------------------------------
            nc.vector.memset(xpad[:], 0.0)
            nc.vector.tensor_copy(xpad[:, 1 : H + 1, 1 : W + 1], xstage[:])

            psums = [
                ppool.tile([128, CHUNK], f32, name=f"ps{f}", tag=f"ps{f}")
                for f in range(NCHUNK)
            ]

            taps = [(kh, kw) for kh in range(K) for kw in range(K)]
            NMIN = 7  # taps[0:NMIN] via DVE min-trick, the rest via ACT abs
            xflat = xpad[:].rearrange("p h w -> p (h w)")
            YF = W * WP  # 3248: rows kh..kh+55 of the padded plane, flat

            # ---- sum-x accumulation (also the PE warm-up burst) --------
            for f in range(NCHUNK):
                r0 = f * CHUNK_ROWS
                for t in range(NMIN):
                    kh, kw = taps[t]
                    rhs = xpad[:, r0 + kh : r0 + kh + CHUNK_ROWS, kw : kw + W]
                    for g in range(4):
                        nc.tensor.matmul(
                            psums[f][32 * g : 32 * g + 32, :],
                            statn[:],
                            rhs,
                            start=(t == 0),
                            stop=False,
                            tile_position=(0, 32 * g),
                        )

            # ---- main loop -------------------------------------------
            # DVE tiles: y = min(x, w)        -> stationary +2
            # ACT tiles: y = |x - w|          -> stationary -1
            # Both computed flat over padded rows kh..kh+55 (contiguous,
            # aligned); the matmul moving-view applies the (kh, kw) shift.
            # ACT taps emitted first within each cc block: ScalarE runs one
            # cc ahead (separate pool), so its tiles are ready when the PE
            # stream reaches the act matmuls.
            tap_order = list(range(NMIN, len(taps))) + list(range(NMIN))
            for cc in range(16):
                for t in tap_order:
                    kh, kw = taps[t]
                    for g in range(4):
                        co = 16 * g + cc
                        src = xflat[:, kh * WP : kh * WP + YF]
                        if t < NMIN:
                            y = ypool.tile([128, YF], bf16, tag="y")
                            nc.vector.tensor_scalar(
                                y[:],
                                src,
                                wbias[:, co, t : t + 1],
                                None,
                                op0=mybir.AluOpType.min,
                            )
                            lhsT = stat2[:, cc, :]
                        else:
                            y = apool.tile([128, YF], bf16, tag="ya")
                            nc.scalar.activation(
                                y[:],
                                src,
                                mybir.ActivationFunctionType.Abs,
                                bias=wneg[:, co, t : t + 1],
                            )
                            lhsT = statm[:, cc, :]
                        y3 = y[:].rearrange("p (r c) -> p r c", c=WP)
                        for f in range(NCHUNK):
                            r0 = f * CHUNK_ROWS
                            rhs = y3[:, r0 : r0 + CHUNK_ROWS, kw : kw + W]
                            nc.tensor.matmul(
                                psums[f][32 * g : 32 * g + 32, :],
                                lhsT,
                                rhs,
                                start=False,
                                stop=(cc == 15 and t == NMIN - 1),
                                tile_position=(0, 32 * g),
                            )

            # ---- epilogue: out = psum + (-sum w), psum -> sbuf -> dram --
            for f in range(NCHUNK):
                nc.scalar.activation(
                    out_sb[:, f * CHUNK : (f + 1) * CHUNK],
                    psums[f][:],
                    mybir.ActivationFunctionType.Identity,
                    bias=swn[:],
                )
            nc.sync.dma_start(o_d.ap(), out_sb[:])

    nc.compile()
    return nc


def get_nc():
    global _nc_cache
    if _nc_cache is None:
        _nc_cache = build_nc()
    return _nc_cache


def make_in_maps(x, w):
    x = np.ascontiguousarray(x, dtype=np.float32)
    w = np.ascontiguousarray(w, dtype=np.float32)
    # -sum w[co] over the min-trick taps (taps[0:7] in kh*3+kw order),
    # scattered to psum partitions p = 32*(co//16)+2*(co%16)+img
    swc = -w.reshape(CO, C, K * K)[:, :, :7].sum(axis=(1, 2))
    swn = np.empty((128, 1), dtype=np.float32)
    for co in range(CO):
        p = 32 * (co // 16) + 2 * (co % 16)
        swn[p, 0] = swc[co]
        swn[p + 1, 0] = swc[co]
    return [
        {"x": x[i * NLOC : (i + 1) * NLOC], "w": w, "swn": swn}
        for i in range(NCORES)
    ]


def unscramble(core_out):
    """[128, L] with row p = 32*(co//16) + 2*(co%16) + img -> [2, 64, 56, 56]."""
    return (
        core_out.reshape(4, 16, NLOC, H, W)
        .transpose(2, 0, 1, 3, 4)
        .reshape(NLOC, CO, H, W)
    )


def kernel(x, w):
    nc = get_nc()
    res = run_bass_kernel_spmd(nc, make_in_maps(x, w), core_ids=list(range(NCORES)))
    out = np.concatenate([unscramble(r["out"]) for r in res.results], axis=0)
    return np.ascontiguousarray(out, dtype=np.float32)


if __name__ == "__main__":
    x = np.random.randn(N, C, H, W).astype(np.float32)
    w = np.random.randn(CO, C, K, K).astype(np.float32)
    o = kernel(x, w)
    print("out", o.shape, o.dtype, float(o.mean()))


# revision 26
# speedup vs baseline: 1.6536x; 1.1362x over previous
"""AdderNet 2D convolution on 8 TRN2 NeuronCores.

out[n,co,h,w] = -sum_{ci,kh,kw} |xpad[n,ci,h+kh,w+kw] - w[co,ci,kh,kw]|

Sharding: data-parallel over the batch dim (16 images -> 2 per core),
weight replicated.  No collectives needed (forward pass only).

Math: |x - w| = x + w - 2*min(x, w), so

  -sum |x - w| = 2*sum min(x, w) - sum x - sum w

The heavy term is one single-op DVE tensor_scalar per (co, tap):
y = min(xpad, w[co,ci,kh,kw]) in bf16 (4x perf mode), evaluated over the
full padded plane so every instruction is contiguous/aligned.  TensorE
reduces partitions with a constant +2 block stationary into PSUM
(accumulating the 9 taps); the (kh,kw) tap shift is applied by the
matmul's strided moving-view.  "sum x" is accumulated by 252 extra
matmuls with an all-(-1) stationary; "sum w" comes in as a tiny
host-precomputed per-partition bias, applied in the epilogue.

Per-core layout:
  - 128 SBUF partitions = img*64 + ci  (2 images per core)
  - psum/output partition p = 32*(co//16) + 2*(co%16) + img
    (TensorE column-tiling: 4 strips of 32, one per co-group)
  - zero padding in xpad contributes min(0, w) terms and the matching
    zeros in sum x, exactly reproducing the reference's |0 - w| border
    terms.
"""

import numpy as np

try:
    from concourse import bacc, mybir, tile
except ImportError:  # pragma: no cover - fallback when sitecustomize absent
    import sys

    sys.path.insert(0, "/opt/trn_rl_repo")
    from concourse import bacc, mybir, tile

from concourse.bass_utils import run_bass_kernel_spmd

N, C, H, W = 16, 64, 56, 56
CO, K = 64, 3
NCORES = 8
NLOC = N // NCORES  # images per core = 2
HP = H + 2  # padded plane height
WP = W + 2
L = H * W  # 3136 output pixels
CHUNK_ROWS = 8  # output rows per psum bank chunk
NCHUNK = H // CHUNK_ROWS  # 7
CHUNK = CHUNK_ROWS * W  # 448 <= 512 fp32 / psum bank

_nc_cache = None


def build_nc():
    nc = bacc.Bacc(
        "TRN2",
        target_bir_lowering=False,
        debug=False,
        num_devices=NCORES,
    )
    f32 = mybir.dt.float32
    bf16 = mybir.dt.bfloat16

    x_d = nc.dram_tensor("x", [NLOC, C, H, W], f32, kind="ExternalInput")
    w_d = nc.dram_tensor("w", [CO, C, K, K], f32, kind="ExternalInput")
    # swn[p, 0] = -sum_{ci,kh,kw} w[co(p)] at psum partition p (host-computed)
    swn_d = nc.dram_tensor("swn", [128, 1], f32, kind="ExternalInput")
    # out rows are psum-partition-major: p = 32*(co//16) + 2*(co%16) + img;
    # the host-side gather untangles this ordering (cheap numpy transpose).
    o_d = nc.dram_tensor("out", [128, L], f32, kind="ExternalOutput")

    with tile.TileContext(nc) as tc:
        with (
            tc.tile_pool(name="const", bufs=1) as cpool,
            tc.tile_pool(name="ypool", bufs=8) as ypool,
            tc.tile_pool(name="apool", bufs=12) as apool,
            tc.tile_pool(name="psum", bufs=1, space="PSUM") as ppool,
        ):
            xstage = cpool.tile([128, H, W], f32)
            xpad = cpool.tile([128, HP, WP], bf16)
            wbias = cpool.tile([128, CO, K * K], f32)
            wneg = cpool.tile([128, CO, K * K], f32)
            swn = cpool.tile([128, 1], f32)
            # stat2[:, c, :]: [128, 32] stationary, col 2c+i = +2 on the
            # img-i partition half, else 0  (the 2*min reduction).
            stat2 = cpool.tile([128, 16, 32], bf16)
            # statm[:, c, :]: same pattern with -1 (the -|x-w| ACT tiles).
            statm = cpool.tile([128, 16, 32], bf16)
            # statn: [128, 32] all-columns -1 on matching img half (sum-x).
            statn = cpool.tile([128, 32], bf16)
            # statx6: like statn but only for cc%4 != 0 pairs (tap 6 is a
            # min-tap only for those co; cc%4==0 co's run tap 6 on ACT).
            statx6 = cpool.tile([128, 32], bf16)
            out_sb = cpool.tile([128, L], f32)

            # ---- loads -------------------------------------------------
            nc.sync.dma_start(xstage[:], x_d.ap().rearrange("n c h w -> (n c) h w"))
            # wbias[p = img*64 + ci, co, kh*3 + kw] = w[co, ci, kh, kw]
            wv = w_d.ap().rearrange("co ci kh kw -> ci co (kh kw)")
            nc.sync.dma_start(wbias[0:64], wv)
            nc.sync.dma_start(wbias[64:128], wv)
            nc.sync.dma_start(swn[:], swn_d.ap())

            # ---- constants --------------------------------------------
            nc.vector.memset(stat2[:], 0.0)
            nc.vector.memset(statm[:], 0.0)
            for c in range(16):
                nc.vector.memset(stat2[0:64, c, 2 * c : 2 * c + 1], 2.0)
                nc.vector.memset(stat2[64:128, c, 2 * c + 1 : 2 * c + 2], 2.0)
                nc.vector.memset(statm[0:64, c, 2 * c : 2 * c + 1], -1.0)
                nc.vector.memset(statm[64:128, c, 2 * c + 1 : 2 * c + 2], -1.0)
            nc.vector.memset(statn[:], 0.0)
            nc.vector.memset(statn[0:64, 0:32:2], -1.0)
            nc.vector.memset(statn[64:128, 1:32:2], -1.0)
            nc.vector.memset(statx6[:], 0.0)
            for c in range(16):
                if c % 4 != 0:
                    nc.vector.memset(statx6[0:64, 2 * c : 2 * c + 1], -1.0)
                    nc.vector.memset(statx6[64:128, 2 * c + 1 : 2 * c + 2], -1.0)
            nc.vector.tensor_scalar(
                wneg[:], wbias[:], -1.0, None, op0=mybir.AluOpType.mult
            )

            # ---- pad + cast to bf16 -----------------------------------
            nc.vector.memset(xpad[:], 0.0)
            nc.vector.tensor_copy(xpad[:, 1 : H + 1, 1 : W + 1], xstage[:])

            psums = [
                ppool.tile([128, CHUNK], f32, name=f"ps{f}", tag=f"ps{f}")
                for f in range(NCHUNK)
            ]

            taps = [(kh, kw) for kh in range(K) for kw in range(K)]
            # taps 7,8 always on ACT; tap 6 on ACT for cc%4==0 co's too
            # (load-balances DVE ~1.2us/tile vs ACT ~3.6us/tile).
            act_taps = lambda cc: (6, 7, 8) if cc % 4 == 0 else (7, 8)
            xflat = xpad[:].rearrange("p h w -> p (h w)")
            YF = W * WP  # 3248: rows kh..kh+55 of the padded plane, flat

            # ---- sum-x accumulation (also the PE warm-up burst) --------
            for f in range(NCHUNK):
                r0 = f * CHUNK_ROWS
                for t in range(7):
                    kh, kw = taps[t]
                    rhs = xpad[:, r0 + kh : r0 + kh + CHUNK_ROWS, kw : kw + W]
                    lhsT = statx6[:] if t == 6 else statn[:]
                    for g in range(4):
                        nc.tensor.matmul(
                            psums[f][32 * g : 32 * g + 32, :],
                            lhsT,
                            rhs,
                            start=(t == 0),
                            stop=False,
                            tile_position=(0, 32 * g),
                        )

            # ---- main loop -------------------------------------------
            # DVE tiles: y = min(x, w)        -> stationary +2
            # ACT tiles: y = |x - w|          -> stationary -1
            # Both computed flat over padded rows kh..kh+55 (contiguous,
            # aligned); the matmul moving-view applies the (kh, kw) shift.
            # ACT taps emitted first within each cc block: ScalarE runs one
            # cc ahead (separate pool), so its tiles are ready when the PE
            # stream reaches the act matmuls.
            for cc in range(16):
                at = act_taps(cc)
                tap_order = list(at) + [t for t in range(9) if t not in at]
                for t in tap_order:
                    kh, kw = taps[t]
                    is_act = t in at
                    for g in range(4):
                        co = 16 * g + cc
                        src = xflat[:, kh * WP : kh * WP + YF]
                        if not is_act:
                            y = ypool.tile([128, YF], bf16, tag="y")
                            nc.vector.tensor_scalar(
                                y[:],
                                src,
                                wbias[:, co, t : t + 1],
                                None,
                                op0=mybir.AluOpType.min,
                            )
                            lhsT = stat2[:, cc, :]
                        else:
                            y = apool.tile([128, YF], bf16, tag="ya")
                            nc.scalar.activation(
                                y[:],
                                src,
                                mybir.ActivationFunctionType.Abs,
                                bias=wneg[:, co, t : t + 1],
                            )
                            lhsT = statm[:, cc, :]
                        y3 = y[:].rearrange("p (r c) -> p r c", c=WP)
                        for f in range(NCHUNK):
                            r0 = f * CHUNK_ROWS
                            rhs = y3[:, r0 : r0 + CHUNK_ROWS, kw : kw + W]
                            nc.tensor.matmul(
                                psums[f][32 * g : 32 * g + 32, :],
                                lhsT,
                                rhs,
                                start=False,
                                stop=(cc == 15 and t == tap_order[-1]),
                                tile_position=(0, 32 * g),
                            )

            # ---- epilogue: out = psum + (-sum w), psum -> sbuf -> dram --
            for f in range(NCHUNK):
                nc.scalar.activation(
                    out_sb[:, f * CHUNK : (f + 1) * CHUNK],
                    psums[f][:],
                    mybir.ActivationFunctionType.Identity,
                    bias=swn[:],
                )
            nc.sync.dma_start(o_d.ap(), out_sb[:])

    nc.compile()
    return nc


def get_nc():
    global _nc_cache
    if _nc_cache is None:
        _nc_cache = build_nc()
    return _nc_cache


def make_in_maps(x, w):
    x = np.ascontiguousarray(x, dtype=np.float32)
    w = np.ascontiguousarray(w, dtype=np.float32)
    # -sum w[co] over each co's min-trick taps (taps 0..5 always; tap 6
    # only when (co%16)%4 != 0), scattered to psum partitions
    # p = 32*(co//16)+2*(co%16)+img
    wr = w.reshape(CO, C, K * K)
    swc = -wr[:, :, :6].sum(axis=(1, 2))
    mask6 = np.array([(co % 16) % 4 != 0 for co in range(CO)])
    swc -= np.where(mask6, wr[:, :, 6].sum(axis=1), 0.0)
    swn = np.empty((128, 1), dtype=np.float32)
    for co in range(CO):
        p = 32 * (co // 16) + 2 * (co % 16)
        swn[p, 0] = swc[co]
        swn[p + 1, 0] = swc[co]
    return [
        {"x": x[i * NLOC : (i + 1) * NLOC], "w": w, "swn": swn}
        for i in range(NCORES)
    ]


def unscramble(core_out):
    """[128, L] with row p = 32*(co//16) + 2*(co%16) + img -> [2, 64, 56, 56]."""
    return (
        core_out.reshape(4, 16, NLOC, H, W)
        .transpose(2, 0, 1, 3, 4)
        .reshape(NLOC, CO, H, W)
    )


def kernel(x, w):
    nc = get_nc()
    res = run_bass_kernel_spmd(nc, make_in_maps(x, w), core_ids=list(range(NCORES)))
    out = np.concatenate([unscramble(r["out"]) for r in res.results], axis=0)
    return np.ascontiguousarray(out, dtype=np.float32)


if __name__ == "__main__":
    x = np.random.randn(N, C, H, W).astype(np.float32)
    w = np.random.randn(CO, C, K, K).astype(np.float32)
    o = kernel(x, w)
    print("out", o.shape, o.dtype, float(o.mean()))


# revision 33
# speedup vs baseline: 1.6720x; 1.0111x over previous
"""AdderNet 2D convolution on 8 TRN2 NeuronCores.

out[n,co,h,w] = -sum_{ci,kh,kw} |xpad[n,ci,h+kh,w+kw] - w[co,ci,kh,kw]|

Sharding: data-parallel over the batch dim (16 images -> 2 per core),
weight replicated.  No collectives needed (forward pass only).

Math: |x - w| = x + w - 2*min(x, w), so

  -sum |x - w| = 2*sum min(x, w) - sum x - sum w

The heavy term is one single-op DVE tensor_scalar per (co, tap):
y = min(xpad, w[co,ci,kh,kw]) in bf16 (4x perf mode), evaluated over the
full padded plane so every instruction is contiguous/aligned.  TensorE
reduces partitions with a constant +2 block stationary into PSUM
(accumulating the 9 taps); the (kh,kw) tap shift is applied by the
matmul's strided moving-view.  "sum x" is accumulated by 252 extra
matmuls with an all-(-1) stationary; "sum w" comes in as a tiny
host-precomputed per-partition bias, applied in the epilogue.

Per-core layout:
  - 128 SBUF partitions = img*64 + ci  (2 images per core)
  - psum/output partition p = 32*(co//16) + 2*(co%16) + img
    (TensorE column-tiling: 4 strips of 32, one per co-group)
  - zero padding in xpad contributes min(0, w) terms and the matching
    zeros in sum x, exactly reproducing the reference's |0 - w| border
    terms.
"""

import numpy as np

try:
    from concourse import bacc, mybir, tile
except ImportError:  # pragma: no cover - fallback when sitecustomize absent
    import sys

    sys.path.insert(0, "/opt/trn_rl_repo")
    from concourse import bacc, mybir, tile

from concourse.bass_utils import run_bass_kernel_spmd

N, C, H, W = 16, 64, 56, 56
CO, K = 64, 3
NCORES = 8
NLOC = N // NCORES  # images per core = 2
HP = H + 2  # padded plane height
WP = W + 2
L = H * W  # 3136 output pixels
CHUNK_ROWS = 8  # output rows per psum bank chunk
NCHUNK = H // CHUNK_ROWS  # 7
CHUNK = CHUNK_ROWS * W  # 448 <= 512 fp32 / psum bank

_nc_cache = None


def build_nc():
    nc = bacc.Bacc(
        "TRN2",
        target_bir_lowering=False,
        debug=False,
        num_devices=NCORES,
    )
    f32 = mybir.dt.float32
    bf16 = mybir.dt.bfloat16

    x_d = nc.dram_tensor("x", [NLOC, C, H, W], f32, kind="ExternalInput")
    w_d = nc.dram_tensor("w", [CO, C, K, K], f32, kind="ExternalInput")
    # swn[p, 0] = -sum_{ci,kh,kw} w[co(p)] at psum partition p (host-computed)
    swn_d = nc.dram_tensor("swn", [128, 1], f32, kind="ExternalInput")
    # out rows are psum-partition-major: p = 32*(co//16) + 2*(co%16) + img;
    # the host-side gather untangles this ordering (cheap numpy transpose).
    o_d = nc.dram_tensor("out", [128, L], f32, kind="ExternalOutput")

    with tile.TileContext(nc) as tc:
        with (
            tc.tile_pool(name="const", bufs=1) as cpool,
            tc.tile_pool(name="ypool", bufs=8) as ypool,
            tc.tile_pool(name="apool", bufs=12) as apool,
            tc.tile_pool(name="psum", bufs=1, space="PSUM") as ppool,
        ):
            xstage = cpool.tile([128, H, W], f32)
            xpad = cpool.tile([128, HP, WP], bf16)
            # xpadB[p, r, c] = xpad[p, r, c+1]: left-shifted copy so the
            # kw==1 DVE taps read from a 4-byte-aligned window (keeps the
            # tensor_scalar in 4x perf mode).
            xpadB = cpool.tile([128, HP, WP], bf16)
            wbias = cpool.tile([128, CO, K * K], f32)
            wneg = cpool.tile([128, CO, K * K], f32)
            swn = cpool.tile([128, 1], f32)
            # stat2[:, c, :]: [128, 32] stationary, col 2c+i = +2 on the
            # img-i partition half, else 0  (the 2*min reduction).
            stat2 = cpool.tile([128, 16, 32], bf16)
            # statm[:, c, :]: same pattern with -1 (the -|x-w| ACT tiles).
            statm = cpool.tile([128, 16, 32], bf16)
            # statn: [128, 32] all-columns -1 on matching img half (sum-x).
            statn = cpool.tile([128, 32], bf16)
            # statx6: like statn but only for cc not in {0,8} pairs (tap 6
            # is a min-tap only for those co; cc in {0,8} run tap 6 on ACT).
            statx6 = cpool.tile([128, 32], bf16)
            out_sb = cpool.tile([128, L], f32)

            # ---- loads -------------------------------------------------
            nc.sync.dma_start(xstage[:], x_d.ap().rearrange("n c h w -> (n c) h w"))
            # wbias[p = img*64 + ci, co, kh*3 + kw] = w[co, ci, kh, kw]
            wv = w_d.ap().rearrange("co ci kh kw -> ci co (kh kw)")
            nc.sync.dma_start(wbias[0:64], wv)
            nc.sync.dma_start(wbias[64:128], wv)
            nc.sync.dma_start(swn[:], swn_d.ap())

            # ---- constants --------------------------------------------
            nc.vector.memset(stat2[:], 0.0)
            nc.vector.memset(statm[:], 0.0)
            for c in range(16):
                nc.vector.memset(stat2[0:64, c, 2 * c : 2 * c + 1], 2.0)
                nc.vector.memset(stat2[64:128, c, 2 * c + 1 : 2 * c + 2], 2.0)
                nc.vector.memset(statm[0:64, c, 2 * c : 2 * c + 1], -1.0)
                nc.vector.memset(statm[64:128, c, 2 * c + 1 : 2 * c + 2], -1.0)
            nc.vector.memset(statn[:], 0.0)
            nc.vector.memset(statn[0:64, 0:32:2], -1.0)
            nc.vector.memset(statn[64:128, 1:32:2], -1.0)
            nc.vector.memset(statx6[:], 0.0)
            for c in range(16):
                if c not in (0, 8):
                    nc.vector.memset(statx6[0:64, 2 * c : 2 * c + 1], -1.0)
                    nc.vector.memset(statx6[64:128, 2 * c + 1 : 2 * c + 2], -1.0)
            nc.vector.tensor_scalar(
                wneg[:], wbias[:], -1.0, None, op0=mybir.AluOpType.mult
            )

            # ---- pad + cast to bf16 -----------------------------------
            nc.vector.memset(xpad[:], 0.0)
            nc.vector.tensor_copy(xpad[:, 1 : H + 1, 1 : W + 1], xstage[:])
            xpf = xpad[:].rearrange("p h w -> p (h w)")
            xbf = xpadB[:].rearrange("p h w -> p (h w)")
            nc.vector.tensor_copy(xbf[:, 0 : HP * WP - 1], xpf[:, 1 : HP * WP])

            psums = [
                ppool.tile([128, CHUNK], f32, name=f"ps{f}", tag=f"ps{f}")
                for f in range(NCHUNK)
            ]

            taps = [(kh, kw) for kh in range(K) for kw in range(K)]
            # taps 7,8 always on ACT; tap 6 on ACT for cc in {0,8} co's too
            # (load-balances DVE ~1.07us/tile vs ACT ~3.5us/tile).
            act_taps = lambda cc: (6, 7, 8) if cc in (0, 8) else (7, 8)

            def tap_src(t):
                """[128, 56, 56] window of the padded plane for tap t,
                4B-aligned for the DVE (odd kw reads the shifted copy)."""
                kh, kw = taps[t]
                if kw == 1:
                    return xpadB[:, kh : kh + H, 0:W]
                return xpad[:, kh : kh + H, kw : kw + W]

            # start-flag bookkeeping: first MM to touch each (g, chunk)
            # PSUM region must carry start=True (emission order == PE order)
            started = [[False] * NCHUNK for _ in range(4)]

            def mm(f, g, lhsT, rhs, stop=False):
                st = not started[g][f]
                started[g][f] = True
                nc.tensor.matmul(
                    psums[f][32 * g : 32 * g + 32, :],
                    lhsT,
                    rhs,
                    start=st,
                    stop=stop,
                    tile_position=(0, 32 * g),
                )

            # sum-x matmul groups, spread through the cc blocks below
            sumx_groups = [(t, f) for t in range(7) for f in range(NCHUNK)]

            def emit_sumx(t, f):
                kh, kw = taps[t]
                r0 = f * CHUNK_ROWS
                rhs = xpad[:, r0 + kh : r0 + kh + CHUNK_ROWS, kw : kw + W]
                lhsT = statx6[:] if t == 6 else statn[:]
                for g in range(4):
                    mm(f, g, lhsT, rhs)

            # ---- main loop -------------------------------------------
            # DVE tiles: y = min(x, w)   (contiguous output) -> stationary +2
            # ACT tiles: y = |x - w|                         -> stationary -1
            # ACT taps emitted first within each cc block: ScalarE runs one
            # cc ahead (separate pool), so its tiles are ready when the PE
            # stream reaches the act matmuls.
            for cc in range(16):
                lo = (len(sumx_groups) * cc) // 16
                hi = (len(sumx_groups) * (cc + 1)) // 16
                for t, f in sumx_groups[lo:hi]:
                    emit_sumx(t, f)
                at = act_taps(cc)
                tap_order = list(at) + [t for t in range(9) if t not in at]
                for t in tap_order:
                    is_act = t in at
                    for g in range(4):
                        co = 16 * g + cc
                        src = tap_src(t)
                        if not is_act:
                            y = ypool.tile([128, L], bf16, tag="y")
                            nc.vector.tensor_scalar(
                                y[:].rearrange("p (a b) -> p a b", b=W),
                                src,
                                wbias[:, co, t : t + 1],
                                None,
                                op0=mybir.AluOpType.min,
                            )
                            lhsT = stat2[:, cc, :]
                        else:
                            y = apool.tile([128, L], bf16, tag="ya")
                            nc.scalar.activation(
                                y[:].rearrange("p (a b) -> p a b", b=W),
                                src,
                                mybir.ActivationFunctionType.Abs,
                                bias=wneg[:, co, t : t + 1],
                            )
                            lhsT = statm[:, cc, :]
                        for f in range(NCHUNK):
                            rhs = y[:, f * CHUNK : (f + 1) * CHUNK]
                            mm(
                                f, g, lhsT, rhs,
                                stop=(cc == 15 and t == tap_order[-1]),
                            )

            # ---- epilogue: out = psum + (-sum w), psum -> sbuf -> dram --
            for f in range(NCHUNK):
                nc.scalar.activation(
                    out_sb[:, f * CHUNK : (f + 1) * CHUNK],
                    psums[f][:],
                    mybir.ActivationFunctionType.Identity,
                    bias=swn[:],
                )
            nc.sync.dma_start(o_d.ap(), out_sb[:])

    nc.compile()
    return nc


def get_nc():
    global _nc_cache
    if _nc_cache is None:
        _nc_cache = build_nc()
    return _nc_cache


def make_in_maps(x, w):
    x = np.ascontiguousarray(x, dtype=np.float32)
    w = np.ascontiguousarray(w, dtype=np.float32)
    # -sum w[co] over each co's min-trick taps (taps 0..5 always; tap 6
    # only when co%16 not in {0,8}), scattered to psum partitions
    # p = 32*(co//16)+2*(co%16)+img
    wr = w.reshape(CO, C, K * K)
    swc = -wr[:, :, :6].sum(axis=(1, 2))
    mask6 = np.array([(co % 16) not in (0, 8) for co in range(CO)])
    swc -= np.where(mask6, wr[:, :, 6].sum(axis=1), 0.0)
    swn = np.empty((128, 1), dtype=np.float32)
    for co in range(CO):
        p = 32 * (co // 16) + 2 * (co % 16)
        swn[p, 0] = swc[co]
        swn[p + 1, 0] = swc[co]
    return [
        {"x": x[i * NLOC : (i + 1) * NLOC], "w": w, "swn": swn}
        for i in range(NCORES)
    ]


def unscramble(core_out):
    """[128, L] with row p = 32*(co//16) + 2*(co%16) + img -> [2, 64, 56, 56]."""
    return (
        core_out.reshape(4, 16, NLOC, H, W)
        .transpose(2, 0, 1, 3, 4)
        .reshape(NLOC, CO, H, W)
    )


def kernel(x, w):
    nc = get_nc()
    res = run_bass_kernel_spmd(nc, make_in_maps(x, w), core_ids=list(range(NCORES)))
    out = np.concatenate([unscramble(r["out"]) for r in res.results], axis=0)
    return np.ascontiguousarray(out, dtype=np.float32)


if __name__ == "__main__":
    x = np.random.randn(N, C, H, W).astype(np.float32)
    w = np.random.randn(CO, C, K, K).astype(np.float32)
    o = kernel(x, w)
    print("out", o.shape, o.dtype, float(o.mean()))


# revision 43
# speedup vs baseline: 1.6915x; 1.0117x over previous
"""AdderNet 2D convolution on 8 TRN2 NeuronCores.

out[n,co,h,w] = -sum_{ci,kh,kw} |xpad[n,ci,h+kh,w+kw] - w[co,ci,kh,kw]|

Sharding: data-parallel over the batch dim (16 images -> 2 per core),
weight replicated.  No collectives needed (forward pass only).

Math: |x - w| = x + w - 2*min(x, w), so

  -sum |x - w| = 2*sum min(x, w) - sum x - sum w

The heavy term is one single-op DVE tensor_scalar per (co, tap):
y = min(xpad, w[co,ci,kh,kw]) in bf16 (4x perf mode), evaluated over the
full padded plane so every instruction is contiguous/aligned.  TensorE
reduces partitions with a constant +2 block stationary into PSUM
(accumulating the 9 taps); the (kh,kw) tap shift is applied by the
matmul's strided moving-view.  "sum x" is accumulated by 252 extra
matmuls with an all-(-1) stationary; "sum w" comes in as a tiny
host-precomputed per-partition bias, applied in the epilogue.

Per-core layout:
  - 128 SBUF partitions = img*64 + ci  (2 images per core)
  - psum/output partition p = 32*(co//16) + 2*(co%16) + img
    (TensorE column-tiling: 4 strips of 32, one per co-group)
  - zero padding in xpad contributes min(0, w) terms and the matching
    zeros in sum x, exactly reproducing the reference's |0 - w| border
    terms.
"""

import numpy as np

try:
    from concourse import bacc, mybir, tile
except ImportError:  # pragma: no cover - fallback when sitecustomize absent
    import sys

    sys.path.insert(0, "/opt/trn_rl_repo")
    from concourse import bacc, mybir, tile

from concourse.bass_utils import run_bass_kernel_spmd

N, C, H, W = 16, 64, 56, 56
CO, K = 64, 3
NCORES = 8
NLOC = N // NCORES  # images per core = 2
HP = H + 2  # padded plane height
WP = W + 2
L = H * W  # 3136 output pixels
CHUNK_ROWS = 8  # output rows per psum bank chunk
NCHUNK = H // CHUNK_ROWS  # 7
CHUNK = CHUNK_ROWS * W  # 448 <= 512 fp32 / psum bank

_nc_cache = None


def build_nc():
    nc = bacc.Bacc(
        "TRN2",
        target_bir_lowering=False,
        debug=False,
        num_devices=NCORES,
    )
    f32 = mybir.dt.float32
    bf16 = mybir.dt.bfloat16

    x_d = nc.dram_tensor("x", [NLOC, C, H, W], f32, kind="ExternalInput")
    w_d = nc.dram_tensor("w", [CO, C, K, K], f32, kind="ExternalInput")
    # swn[p, 0] = -sum_{ci,kh,kw} w[co(p)] at psum partition p (host-computed)
    swn_d = nc.dram_tensor("swn", [128, 1], f32, kind="ExternalInput")
    # out rows are psum-partition-major: p = 32*(co//16) + 2*(co%16) + img;
    # the host-side gather untangles this ordering (cheap numpy transpose).
    o_d = nc.dram_tensor("out", [128, L], f32, kind="ExternalOutput")

    with tile.TileContext(nc) as tc:
        with (
            tc.tile_pool(name="const", bufs=1) as cpool,
            tc.tile_pool(name="ypool", bufs=10) as ypool,
            tc.tile_pool(name="apool", bufs=12) as apool,
            tc.tile_pool(name="psum", bufs=1, space="PSUM") as ppool,
        ):
            # padded f32 plane (DMA target; ACT reads it directly)
            xpadf = cpool.tile([128, HP, WP], f32)
            xpad = cpool.tile([128, HP, WP], bf16)
            # xpadB[p, r, c] = xpad[p, r, c+1]: left-shifted copy so the
            # kw==1 DVE taps read from a 4-byte-aligned window (keeps the
            # tensor_scalar in 4x perf mode).
            xpadB = cpool.tile([128, HP, WP], bf16)
            wbias = cpool.tile([128, CO, K * K], f32)
            wneg = cpool.tile([128, CO, K * K], f32)
            swn = cpool.tile([128, 1], f32)
            # stat2[:, c, :]: [128, 32] stationary, col 2c+i = +2 on the
            # img-i partition half, else 0  (the 2*min reduction).
            stat2 = cpool.tile([128, 16, 32], bf16)
            # statm[:, c, :]: same pattern with -1 (the -|x-w| ACT tiles).
            statm = cpool.tile([128, 16, 32], bf16)
            # statn: [128, 32] all-columns -1 on matching img half (sum-x).
            statn = cpool.tile([128, 32], bf16)
            # statx6: like statn but only for cc not in {0,8} pairs (tap 6
            # is a min-tap only for those co; cc in {0,8} run tap 6 on ACT).
            statx6 = cpool.tile([128, 32], bf16)
            out_sb = cpool.tile([128, L], f32)

            # ---- loads -------------------------------------------------
            # zero only the pad border, then DMA x into the interior
            # (4-way split across DMA queues)
            nc.vector.memset(xpadf[:, 0, :], 0.0)
            nc.vector.memset(xpadf[:, HP - 1, :], 0.0)
            nc.vector.memset(xpadf[:, 1 : H + 1, 0:1], 0.0)
            nc.vector.memset(xpadf[:, 1 : H + 1, WP - 1 : WP], 0.0)
            xv = x_d.ap().rearrange("n c h w -> (n c) h w")
            for q in range(4):
                nc.sync.dma_start(
                    xpadf[32 * q : 32 * (q + 1), 1 : H + 1, 1 : W + 1],
                    xv[32 * q : 32 * (q + 1)],
                )
            # wbias[p = img*64 + ci, co, kh*3 + kw] = w[co, ci, kh, kw]
            wv = w_d.ap().rearrange("co ci kh kw -> ci co (kh kw)")
            nc.sync.dma_start(wbias[0:64], wv)
            nc.sync.dma_start(wbias[64:128], wv)
            nc.sync.dma_start(swn[:], swn_d.ap())

            # ---- constants --------------------------------------------
            nc.vector.memset(stat2[:], 0.0)
            nc.vector.memset(statm[:], 0.0)
            for c in range(16):
                nc.vector.memset(stat2[0:64, c, 2 * c : 2 * c + 1], 2.0)
                nc.vector.memset(stat2[64:128, c, 2 * c + 1 : 2 * c + 2], 2.0)
                nc.vector.memset(statm[0:64, c, 2 * c : 2 * c + 1], -1.0)
                nc.vector.memset(statm[64:128, c, 2 * c + 1 : 2 * c + 2], -1.0)
            nc.vector.memset(statn[:], 0.0)
            nc.vector.memset(statn[0:64, 0:32:2], -1.0)
            nc.vector.memset(statn[64:128, 1:32:2], -1.0)
            nc.vector.memset(statx6[:], 0.0)
            for c in range(16):
                if c not in (0, 5, 8, 13):
                    nc.vector.memset(statx6[0:64, 2 * c : 2 * c + 1], -1.0)
                    nc.vector.memset(statx6[64:128, 2 * c + 1 : 2 * c + 2], -1.0)
            nc.scalar.activation(
                wneg[:], wbias[:], mybir.ActivationFunctionType.Identity,
                scale=-1.0,
            )

            # ---- cast to bf16 (+ shifted copy) ------------------------
            xpf = xpad[:].rearrange("p h w -> p (h w)")
            xbf = xpadB[:].rearrange("p h w -> p (h w)")
            nc.vector.tensor_copy(xpf[:], xpadf[:].rearrange("p h w -> p (h w)"))
            nc.vector.tensor_copy(xbf[:, 0 : HP * WP - 1], xpf[:, 1 : HP * WP])
            wb2 = wbias[:].rearrange("p a b -> p (a b)")
            wn2 = wneg[:].rearrange("p a b -> p (a b)")

            psums = [
                ppool.tile([128, CHUNK], f32, name=f"ps{f}", tag=f"ps{f}")
                for f in range(NCHUNK)
            ]

            taps = [(kh, kw) for kh in range(K) for kw in range(K)]
            # taps 7,8 always on ACT; tap 6 on ACT for cc in ACT3 co's too
            # (load-balances DVE ~1.23us/tile vs ACT ~3.5us/tile).
            ACT3 = (0, 5, 8, 13)
            act_taps = lambda cc: (6, 7, 8) if cc in ACT3 else (7, 8)

            def tap_src(t):
                """[128, 56, 56] window of the padded plane for tap t,
                4B-aligned for the DVE (odd kw reads the shifted copy)."""
                kh, kw = taps[t]
                if kw == 1:
                    return xpadB[:, kh : kh + H, 0:W]
                return xpad[:, kh : kh + H, kw : kw + W]

            def tap_src_f32(t):
                kh, kw = taps[t]
                return xpadf[:, kh : kh + H, kw : kw + W]

            # start-flag bookkeeping: first MM to touch each (g, chunk)
            # PSUM region must carry start=True (emission order == PE order)
            started = [[False] * NCHUNK for _ in range(4)]

            def mm(f, g, lhsT, rhs, stop=False):
                st = not started[g][f]
                started[g][f] = True
                nc.tensor.matmul(
                    psums[f][32 * g : 32 * g + 32, :],
                    lhsT,
                    rhs,
                    start=st,
                    stop=stop,
                    tile_position=(0, 32 * g),
                )

            # sum-x matmul groups, spread through the cc blocks below
            sumx_groups = [(t, f) for t in range(7) for f in range(NCHUNK)]

            def emit_sumx(t, f):
                kh, kw = taps[t]
                r0 = f * CHUNK_ROWS
                rhs = xpad[:, r0 + kh : r0 + kh + CHUNK_ROWS, kw : kw + W]
                lhsT = statx6[:] if t == 6 else statn[:]
                for g in range(4):
                    mm(f, g, lhsT, rhs)

            # ---- main loop -------------------------------------------
            # DVE tiles: y = min(x, w)   (contiguous output) -> stationary +2
            # ACT tiles: y = |x - w|                         -> stationary -1
            # ACT taps emitted first within each cc block: ScalarE runs one
            # cc ahead (separate pool), so its tiles are ready when the PE
            # stream reaches the act matmuls.
            for cc in range(16):
                lo = (len(sumx_groups) * cc) // 16
                hi = (len(sumx_groups) * (cc + 1)) // 16
                for t, f in sumx_groups[lo:hi]:
                    emit_sumx(t, f)
                at = act_taps(cc)
                if cc == 0:
                    # first block: ACT starts ~10us late (DMA+cast deps);
                    # consume its tiles at the END of the block instead.
                    tap_order = [t for t in range(9) if t not in at] + list(at)
                else:
                    tap_order = list(at) + [t for t in range(9) if t not in at]
                for t in tap_order:
                    is_act = t in at
                    for g in range(4):
                        co = 16 * g + cc
                        if not is_act:
                            y = ypool.tile([128, L], bf16, tag="y")
                            nc.vector.tensor_scalar(
                                y[:].rearrange("p (a b) -> p a b", b=W),
                                tap_src(t),
                                wb2[:, co * 9 + t : co * 9 + t + 1],
                                None,
                                op0=mybir.AluOpType.min,
                            )
                            lhsT = stat2[:, cc, :]
                        else:
                            y = apool.tile([128, L], bf16, tag="ya")
                            nc.scalar.activation(
                                y[:].rearrange("p (a b) -> p a b", b=W),
                                tap_src_f32(t),
                                mybir.ActivationFunctionType.Abs,
                                bias=wn2[:, co * 9 + t : co * 9 + t + 1],
                            )
                            lhsT = statm[:, cc, :]
                        for f in range(NCHUNK):
                            rhs = y[:, f * CHUNK : (f + 1) * CHUNK]
                            mm(
                                f, g, lhsT, rhs,
                                stop=(cc == 15 and t == tap_order[-1]),
                            )

            # ---- epilogue: out = psum + (-sum w), psum -> sbuf -> dram --
            for f in range(NCHUNK):
                nc.scalar.activation(
                    out_sb[:, f * CHUNK : (f + 1) * CHUNK],
                    psums[f][:],
                    mybir.ActivationFunctionType.Identity,
                    bias=swn[:],
                )
            for q in range(4):
                nc.sync.dma_start(
                    o_d.ap()[32 * q : 32 * (q + 1), :],
                    out_sb[32 * q : 32 * (q + 1), :],
                )

    nc.compile()
    return nc


def get_nc():
    global _nc_cache
    if _nc_cache is None:
        _nc_cache = build_nc()
    return _nc_cache


def make_in_maps(x, w):
    x = np.ascontiguousarray(x, dtype=np.float32)
    w = np.ascontiguousarray(w, dtype=np.float32)
    # -sum w[co] over each co's min-trick taps (taps 0..5 always; tap 6
    # only when co%16 not in {0,8}), scattered to psum partitions
    # p = 32*(co//16)+2*(co%16)+img
    wr = w.reshape(CO, C, K * K)
    swc = -wr[:, :, :6].sum(axis=(1, 2))
    mask6 = np.array([(co % 16) not in (0, 5, 8, 13) for co in range(CO)])
    swc -= np.where(mask6, wr[:, :, 6].sum(axis=1), 0.0)
    swn = np.empty((128, 1), dtype=np.float32)
    for co in range(CO):
        p = 32 * (co // 16) + 2 * (co % 16)
        swn[p, 0] = swc[co]
        swn[p + 1, 0] = swc[co]
    return [
        {"x": x[i * NLOC : (i + 1) * NLOC], "w": w, "swn": swn}
        for i in range(NCORES)
    ]


def unscramble(core_out):
    """[128, L] with row p = 32*(co//16) + 2*(co%16) + img -> [2, 64, 56, 56]."""
    return (
        core_out.reshape(4, 16, NLOC, H, W)
        .transpose(2, 0, 1, 3, 4)
        .reshape(NLOC, CO, H, W)
    )


def kernel(x, w):
    nc = get_nc()
    res = run_bass_kernel_spmd(nc, make_in_maps(x, w), core_ids=list(range(NCORES)))
    out = np.concatenate([unscramble(r["out"]) for r in res.results], axis=0)
    return np.ascontiguousarray(out, dtype=np.float32)


if __name__ == "__main__":
    x = np.random.randn(N, C, H, W).astype(np.float32)
    w = np.random.randn(CO, C, K, K).astype(np.float32)
    o = kernel(x, w)
    print("out", o.shape, o.dtype, float(o.mean()))


# revision 45
# speedup vs baseline: 1.7257x; 1.0202x over previous
"""AdderNet 2D convolution on 8 TRN2 NeuronCores.

out[n,co,h,w] = -sum_{ci,kh,kw} |xpad[n,ci,h+kh,w+kw] - w[co,ci,kh,kw]|

Sharding: data-parallel over the batch dim (16 images -> 2 per core),
weight replicated.  No collectives needed (forward pass only).

Math: |x - w| = x + w - 2*min(x, w), so

  -sum |x - w| = 2*sum min(x, w) - sum x - sum w

The heavy term is one single-op DVE tensor_scalar per (co, tap):
y = min(xpad, w[co,ci,kh,kw]) in bf16 (4x perf mode), evaluated over the
full padded plane so every instruction is contiguous/aligned.  TensorE
reduces partitions with a constant +2 block stationary into PSUM
(accumulating the 9 taps); the (kh,kw) tap shift is applied by the
matmul's strided moving-view.  "sum x" is accumulated by 252 extra
matmuls with an all-(-1) stationary; "sum w" comes in as a tiny
host-precomputed per-partition bias, applied in the epilogue.

Per-core layout:
  - 128 SBUF partitions = img*64 + ci  (2 images per core)
  - psum/output partition p = 32*(co//16) + 2*(co%16) + img
    (TensorE column-tiling: 4 strips of 32, one per co-group)
  - zero padding in xpad contributes min(0, w) terms and the matching
    zeros in sum x, exactly reproducing the reference's |0 - w| border
    terms.
"""

import numpy as np

try:
    from concourse import bacc, mybir, tile
except ImportError:  # pragma: no cover - fallback when sitecustomize absent
    import sys

    sys.path.insert(0, "/opt/trn_rl_repo")
    from concourse import bacc, mybir, tile

from concourse.bass_utils import run_bass_kernel_spmd

N, C, H, W = 16, 64, 56, 56
CO, K = 64, 3
NCORES = 8
NLOC = N // NCORES  # images per core = 2
HP = H + 2  # padded plane height
WP = W + 2
L = H * W  # 3136 output pixels
CHUNK_ROWS = 8  # output rows per psum bank chunk
NCHUNK = H // CHUNK_ROWS  # 7
CHUNK = CHUNK_ROWS * W  # 448 <= 512 fp32 / psum bank

_nc_cache = None


def build_nc():
    nc = bacc.Bacc(
        "TRN2",
        target_bir_lowering=False,
        debug=False,
        num_devices=NCORES,
    )
    f32 = mybir.dt.float32
    bf16 = mybir.dt.bfloat16

    x_d = nc.dram_tensor("x", [NLOC, C, H, W], f32, kind="ExternalInput")
    # host-preshuffled weights: wb[p=img*64+ci, co*9+t] = w[co, ci, t]
    wb_d = nc.dram_tensor("wb", [128, CO * K * K], f32, kind="ExternalInput")
    wn_d = nc.dram_tensor("wn", [128, CO * K * K], f32, kind="ExternalInput")
    # swn[p, 0] = -sum_{ci, min-taps} w[co(p)] at psum partition p (host side)
    swn_d = nc.dram_tensor("swn", [128, 1], f32, kind="ExternalInput")
    # out rows are psum-partition-major: p = 32*(co//16) + 2*(co%16) + img;
    # the host-side gather untangles this ordering (cheap numpy transpose).
    o_d = nc.dram_tensor("out", [128, L], f32, kind="ExternalOutput")

    with tile.TileContext(nc) as tc:
        with (
            tc.tile_pool(name="const", bufs=1) as cpool,
            tc.tile_pool(name="ypool", bufs=10) as ypool,
            tc.tile_pool(name="apool", bufs=12) as apool,
            tc.tile_pool(name="psum", bufs=1, space="PSUM") as ppool,
        ):
            # padded f32 plane (DMA target; ACT reads it directly)
            xpadf = cpool.tile([128, HP, WP], f32)
            xpad = cpool.tile([128, HP, WP], bf16)
            # xpadB[p, r, c] = xpad[p, r, c+1]: left-shifted copy so the
            # kw==1 DVE taps read from a 4-byte-aligned window (keeps the
            # tensor_scalar in 4x perf mode).
            xpadB = cpool.tile([128, HP, WP], bf16)
            wbias = cpool.tile([128, CO * K * K], f32)
            wneg = cpool.tile([128, CO * K * K], f32)
            swn = cpool.tile([128, 1], f32)
            # stat2[:, c, :]: [128, 32] stationary, col 2c+i = +2 on the
            # img-i partition half, else 0  (the 2*min reduction).
            stat2 = cpool.tile([128, 16, 32], bf16)
            # statm[:, c, :]: same pattern with -1 (the -|x-w| ACT tiles).
            statm = cpool.tile([128, 16, 32], bf16)
            # statn: [128, 32] all-columns -1 on matching img half (sum-x).
            statn = cpool.tile([128, 32], bf16)
            # statx6/statx7: like statn but only over the co columns for
            # which that tap runs on the DVE (min-trick) path.
            statx6 = cpool.tile([128, 32], bf16)
            statx7 = cpool.tile([128, 32], bf16)
            out_sb = cpool.tile([128, L], f32)

            # ---- loads -------------------------------------------------
            # zero only the pad border, then DMA x into the interior
            # (4-way split across DMA queues)
            nc.vector.memset(xpadf[:, 0, :], 0.0)
            nc.vector.memset(xpadf[:, HP - 1, :], 0.0)
            nc.vector.memset(xpadf[:, 1 : H + 1, 0:1], 0.0)
            nc.vector.memset(xpadf[:, 1 : H + 1, WP - 1 : WP], 0.0)
            xv = x_d.ap().rearrange("n c h w -> (n c) h w")
            for q in range(8):
                nc.sync.dma_start(
                    xpadf[16 * q : 16 * (q + 1), 1 : H + 1, 1 : W + 1],
                    xv[16 * q : 16 * (q + 1)],
                )
            nc.sync.dma_start(wbias[:], wb_d.ap())
            nc.sync.dma_start(wneg[:], wn_d.ap())
            nc.sync.dma_start(swn[:], swn_d.ap())

            # ---- constants --------------------------------------------
            nc.vector.memset(stat2[:], 0.0)
            nc.vector.memset(statm[:], 0.0)
            for c in range(16):
                nc.vector.memset(stat2[0:64, c, 2 * c : 2 * c + 1], 2.0)
                nc.vector.memset(stat2[64:128, c, 2 * c + 1 : 2 * c + 2], 2.0)
                nc.vector.memset(statm[0:64, c, 2 * c : 2 * c + 1], -1.0)
                nc.vector.memset(statm[64:128, c, 2 * c + 1 : 2 * c + 2], -1.0)
            nc.vector.memset(statn[:], 0.0)
            nc.vector.memset(statn[0:64, 0:32:2], -1.0)
            nc.vector.memset(statn[64:128, 1:32:2], -1.0)
            nc.vector.memset(statx6[:], 0.0)
            for c in range(16):
                if c not in (2, 5, 8, 11, 14):
                    nc.vector.memset(statx6[0:64, 2 * c : 2 * c + 1], -1.0)
                    nc.vector.memset(statx6[64:128, 2 * c + 1 : 2 * c + 2], -1.0)
            nc.vector.memset(statx7[:], 0.0)
            nc.vector.memset(statx7[0:64, 0:1], -1.0)
            nc.vector.memset(statx7[64:128, 1:2], -1.0)

            # ---- cast to bf16 (+ shifted copy) ------------------------
            xpf = xpad[:].rearrange("p h w -> p (h w)")
            xbf = xpadB[:].rearrange("p h w -> p (h w)")
            nc.vector.tensor_copy(xpf[:], xpadf[:].rearrange("p h w -> p (h w)"))
            nc.vector.tensor_copy(xbf[:, 0 : HP * WP - 1], xpf[:, 1 : HP * WP])
            wb2 = wbias[:]
            wn2 = wneg[:]

            psums = [
                ppool.tile([128, CHUNK], f32, name=f"ps{f}", tag=f"ps{f}")
                for f in range(NCHUNK)
            ]

            taps = [(kh, kw) for kh in range(K) for kw in range(K)]
            # act-tap distribution (total 36 tap-slots = 144 act tiles):
            # cc0 gets a single act tap (8,) so the first PE block is not
            # gated on the ScalarE warm-up; ACT3 blocks get three.
            ACT3 = (2, 5, 8, 11, 14)
            def act_taps(cc):
                if cc == 0:
                    return (8,)
                return (6, 7, 8) if cc in ACT3 else (7, 8)

            def tap_src(t):
                """[128, 56, 56] window of the padded plane for tap t,
                4B-aligned for the DVE (odd kw reads the shifted copy)."""
                kh, kw = taps[t]
                if kw == 1:
                    return xpadB[:, kh : kh + H, 0:W]
                return xpad[:, kh : kh + H, kw : kw + W]

            def tap_src_f32(t):
                kh, kw = taps[t]
                return xpadf[:, kh : kh + H, kw : kw + W]

            # start-flag bookkeeping: first MM to touch each (g, chunk)
            # PSUM region must carry start=True (emission order == PE order)
            started = [[False] * NCHUNK for _ in range(4)]

            def mm(f, g, lhsT, rhs, stop=False):
                st = not started[g][f]
                started[g][f] = True
                nc.tensor.matmul(
                    psums[f][32 * g : 32 * g + 32, :],
                    lhsT,
                    rhs,
                    start=st,
                    stop=stop,
                    tile_position=(0, 32 * g),
                )

            # sum-x matmul groups, spread through the cc blocks below
            sumx_groups = [(t, f) for t in range(8) for f in range(NCHUNK)]

            def emit_sumx(t, f):
                kh, kw = taps[t]
                r0 = f * CHUNK_ROWS
                rhs = xpad[:, r0 + kh : r0 + kh + CHUNK_ROWS, kw : kw + W]
                lhsT = {6: statx6[:], 7: statx7[:]}.get(t, statn[:])
                for g in range(4):
                    mm(f, g, lhsT, rhs)

            # ---- main loop -------------------------------------------
            # DVE tiles: y = min(x, w)   (contiguous output) -> stationary +2
            # ACT tiles: y = |x - w|                         -> stationary -1
            # ACT taps emitted first within each cc block: ScalarE runs one
            # cc ahead (separate pool), so its tiles are ready when the PE
            # stream reaches the act matmuls.
            for cc in range(16):
                lo = (len(sumx_groups) * cc) // 16
                hi = (len(sumx_groups) * (cc + 1)) // 16
                for t, f in sumx_groups[lo:hi]:
                    emit_sumx(t, f)
                at = act_taps(cc)
                if cc == 0:
                    # first block: ACT starts ~10us late (DMA+cast deps);
                    # consume its tiles at the END of the block instead.
                    tap_order = [t for t in range(9) if t not in at] + list(at)
                else:
                    tap_order = list(at) + [t for t in range(9) if t not in at]
                for t in tap_order:
                    is_act = t in at
                    for g in range(4):
                        co = 16 * g + cc
                        if not is_act:
                            y = ypool.tile([128, L], bf16, tag="y")
                            nc.vector.tensor_scalar(
                                y[:].rearrange("p (a b) -> p a b", b=W),
                                tap_src(t),
                                wb2[:, co * 9 + t : co * 9 + t + 1],
                                None,
                                op0=mybir.AluOpType.min,
                            )
                            lhsT = stat2[:, cc, :]
                        else:
                            y = apool.tile([128, L], bf16, tag="ya")
                            nc.scalar.activation(
                                y[:].rearrange("p (a b) -> p a b", b=W),
                                tap_src_f32(t),
                                mybir.ActivationFunctionType.Abs,
                                bias=wn2[:, co * 9 + t : co * 9 + t + 1],
                            )
                            lhsT = statm[:, cc, :]
                        for f in range(NCHUNK):
                            rhs = y[:, f * CHUNK : (f + 1) * CHUNK]
                            mm(
                                f, g, lhsT, rhs,
                                stop=(cc == 15 and t == tap_order[-1]),
                            )

            # ---- epilogue: out = psum + (-sum w), psum -> sbuf -> dram --
            for f in range(NCHUNK):
                nc.scalar.activation(
                    out_sb[:, f * CHUNK : (f + 1) * CHUNK],
                    psums[f][:],
                    mybir.ActivationFunctionType.Identity,
                    bias=swn[:],
                )
            for q in range(4):
                nc.sync.dma_start(
                    o_d.ap()[32 * q : 32 * (q + 1), :],
                    out_sb[32 * q : 32 * (q + 1), :],
                )

    nc.compile()
    return nc


def get_nc():
    global _nc_cache
    if _nc_cache is None:
        _nc_cache = build_nc()
    return _nc_cache


def make_in_maps(x, w):
    x = np.ascontiguousarray(x, dtype=np.float32)
    w = np.ascontiguousarray(w, dtype=np.float32)
    wr = w.reshape(CO, C, K * K)
    # wb[p = img*64 + ci, co*9 + t] = w[co, ci, t]  (both img halves)
    wb_half = wr.transpose(1, 0, 2).reshape(C, CO * K * K)
    wb = np.ascontiguousarray(np.vstack([wb_half, wb_half]), dtype=np.float32)
    wn = np.ascontiguousarray(-wb)
    # -sum w[co] over each co's min-trick taps: t 0..5 always; t6 when
    # cc=co%16 not in ACT3={2,5,8,11,14}; t7 only when cc==0; t8 never.
    # Scattered to psum partitions p = 32*(co//16)+2*(co%16)+img.
    swc = -wr[:, :, :6].sum(axis=(1, 2))
    mask6 = np.array([(co % 16) not in (2, 5, 8, 11, 14) for co in range(CO)])
    swc -= np.where(mask6, wr[:, :, 6].sum(axis=1), 0.0)
    mask7 = np.array([(co % 16) == 0 for co in range(CO)])
    swc -= np.where(mask7, wr[:, :, 7].sum(axis=1), 0.0)
    swn = np.empty((128, 1), dtype=np.float32)
    for co in range(CO):
        p = 32 * (co // 16) + 2 * (co % 16)
        swn[p, 0] = swc[co]
        swn[p + 1, 0] = swc[co]
    return [
        {"x": x[i * NLOC : (i + 1) * NLOC], "wb": wb, "wn": wn, "swn": swn}
        for i in range(NCORES)
    ]


def unscramble(core_out):
    """[128, L] with row p = 32*(co//16) + 2*(co%16) + img -> [2, 64, 56, 56]."""
    return (
        core_out.reshape(4, 16, NLOC, H, W)
        .transpose(2, 0, 1, 3, 4)
        .reshape(NLOC, CO, H, W)
    )


def kernel(x, w):
    nc = get_nc()
    res = run_bass_kernel_spmd(nc, make_in_maps(x, w), core_ids=list(range(NCORES)))
    out = np.concatenate([unscramble(r["out"]) for r in res.results], axis=0)
    return np.ascontiguousarray(out, dtype=np.float32)


if __name__ == "__main__":
    x = np.random.randn(N, C, H, W).astype(np.float32)
    w = np.random.randn(CO, C, K, K).astype(np.float32)
    o = kernel(x, w)
    print("out", o.shape, o.dtype, float(o.mean()))


# revision 47
# speedup vs baseline: 1.9694x; 1.1413x over previous
"""AdderNet 2D convolution on 8 TRN2 NeuronCores.

out[n,co,h,w] = -sum_{ci,kh,kw} |xpad[n,ci,h+kh,w+kw] - w[co,ci,kh,kw]|

Sharding: data-parallel over the batch dim (16 images -> 2 per core),
weight replicated.  No collectives needed (forward pass only).

Math: |x - w| = x + w - 2*min(x, w), so

  -sum |x - w| = 2*sum min(x, w) - sum x - sum w

The heavy term is one single-op DVE tensor_scalar per (co, tap):
y = min(xpad, w[co,ci,kh,kw]) in bf16 (4x perf mode), evaluated over the
full padded plane so every instruction is contiguous/aligned.  TensorE
reduces partitions with a constant +2 block stationary into PSUM
(accumulating the 9 taps); the (kh,kw) tap shift is applied by the
matmul's strided moving-view.  "sum x" is accumulated by 252 extra
matmuls with an all-(-1) stationary; "sum w" comes in as a tiny
host-precomputed per-partition bias, applied in the epilogue.

Per-core layout:
  - 128 SBUF partitions = img*64 + ci  (2 images per core)
  - psum/output partition p = 32*(co//16) + 2*(co%16) + img
    (TensorE column-tiling: 4 strips of 32, one per co-group)
  - zero padding in xpad contributes min(0, w) terms and the matching
    zeros in sum x, exactly reproducing the reference's |0 - w| border
    terms.
"""

import numpy as np

try:
    from concourse import bacc, mybir, tile
except ImportError:  # pragma: no cover - fallback when sitecustomize absent
    import sys

    sys.path.insert(0, "/opt/trn_rl_repo")
    from concourse import bacc, mybir, tile

from concourse.bass_utils import run_bass_kernel_spmd

N, C, H, W = 16, 64, 56, 56
CO, K = 64, 3
NCORES = 8
NLOC = N // NCORES  # images per core = 2
HP = H + 2  # padded plane height
WP = W + 2
L = H * W  # 3136 output pixels
CHUNK_ROWS = 8  # output rows per psum bank chunk
NCHUNK = H // CHUNK_ROWS  # 7
CHUNK = CHUNK_ROWS * W  # 448 <= 512 fp32 / psum bank

_nc_cache = None


def build_nc():
    nc = bacc.Bacc(
        "TRN2",
        target_bir_lowering=False,
        debug=False,
        num_devices=NCORES,
    )
    f32 = mybir.dt.float32
    bf16 = mybir.dt.bfloat16

    x_d = nc.dram_tensor("x", [NLOC, C, H, W], bf16, kind="ExternalInput")
    # host-preshuffled weights: wb[p=img*64+ci, co*9+t] = w[co, ci, t]
    wb_d = nc.dram_tensor("wb", [128, CO * K * K], f32, kind="ExternalInput")
    wn_d = nc.dram_tensor("wn", [128, CO * K * K], f32, kind="ExternalInput")
    # swn[p, 0] = -sum_{ci, min-taps} w[co(p)] at psum partition p (host side)
    swn_d = nc.dram_tensor("swn", [128, 1], f32, kind="ExternalInput")
    # out rows are psum-partition-major: p = 32*(co//16) + 2*(co%16) + img;
    # the host-side gather untangles this ordering (cheap numpy transpose).
    o_d = nc.dram_tensor("out", [128, L], f32, kind="ExternalOutput")

    with tile.TileContext(nc) as tc:
        with (
            tc.tile_pool(name="const", bufs=1) as cpool,
            tc.tile_pool(name="ypool", bufs=10) as ypool,
            tc.tile_pool(name="apool", bufs=16) as apool,
            tc.tile_pool(name="psum", bufs=1, space="PSUM") as ppool,
        ):
            xpad = cpool.tile([128, HP, WP], bf16)
            # xpadB[p, r, c] = xpad[p, r, c+1]: left-shifted copy so the
            # kw==1 DVE taps read from a 4-byte-aligned window (keeps the
            # tensor_scalar in 4x perf mode).
            xpadB = cpool.tile([128, HP, WP], bf16)
            wbias = cpool.tile([128, CO * K * K], f32)
            wneg = cpool.tile([128, CO * K * K], f32)
            swn = cpool.tile([128, 1], f32)
            # stat2[:, c, :]: [128, 32] stationary, col 2c+i = +2 on the
            # img-i partition half, else 0  (the 2*min reduction).
            stat2 = cpool.tile([128, 16, 32], bf16)
            # statm[:, c, :]: same pattern with -1 (the -|x-w| ACT tiles).
            statm = cpool.tile([128, 16, 32], bf16)
            # statn: [128, 32] all-columns -1 on matching img half (sum-x).
            statn = cpool.tile([128, 32], bf16)
            # statx6/statx7: like statn but only over the co columns for
            # which that tap runs on the DVE (min-trick) path.
            statx6 = cpool.tile([128, 32], bf16)
            statx7 = cpool.tile([128, 32], bf16)
            out_sb = cpool.tile([128, L], f32)

            # ---- loads -------------------------------------------------
            # zero only the pad border, then DMA bf16 x into the interior
            # (split across engine DGE rings for queue parallelism)
            nc.vector.memset(xpad[:, 0, :], 0.0)
            nc.vector.memset(xpad[:, HP - 1, :], 0.0)
            nc.vector.memset(xpad[:, 1 : H + 1, 0:1], 0.0)
            nc.vector.memset(xpad[:, 1 : H + 1, WP - 1 : WP], 0.0)
            xv = x_d.ap().rearrange("n c h w -> (n c) h w")
            dma_engs = [nc.sync, nc.gpsimd, nc.scalar, nc.sync,
                        nc.gpsimd, nc.scalar, nc.sync, nc.gpsimd]
            for q in range(8):
                dma_engs[q].dma_start(
                    xpad[16 * q : 16 * (q + 1), 1 : H + 1, 1 : W + 1],
                    xv[16 * q : 16 * (q + 1)],
                )
            nc.sync.dma_start(wbias[:], wb_d.ap())
            nc.gpsimd.dma_start(wneg[:], wn_d.ap())
            nc.sync.dma_start(swn[:], swn_d.ap())

            # ---- constants --------------------------------------------
            nc.vector.memset(stat2[:], 0.0)
            nc.vector.memset(statm[:], 0.0)
            for c in range(16):
                nc.vector.memset(stat2[0:64, c, 2 * c : 2 * c + 1], 2.0)
                nc.vector.memset(stat2[64:128, c, 2 * c + 1 : 2 * c + 2], 2.0)
                nc.vector.memset(statm[0:64, c, 2 * c : 2 * c + 1], -1.0)
                nc.vector.memset(statm[64:128, c, 2 * c + 1 : 2 * c + 2], -1.0)
            nc.vector.memset(statn[:], 0.0)
            nc.vector.memset(statn[0:64, 0:32:2], -1.0)
            nc.vector.memset(statn[64:128, 1:32:2], -1.0)
            nc.vector.memset(statx6[:], 0.0)
            for c in range(16):
                if c not in (2, 4, 7, 9, 11, 14):
                    nc.vector.memset(statx6[0:64, 2 * c : 2 * c + 1], -1.0)
                    nc.vector.memset(statx6[64:128, 2 * c + 1 : 2 * c + 2], -1.0)
            nc.vector.memset(statx7[:], 0.0)
            nc.vector.memset(statx7[0:64, 0:1], -1.0)
            nc.vector.memset(statx7[64:128, 1:2], -1.0)

            # ---- shifted copy for the odd-kw taps ---------------------
            xpf = xpad[:].rearrange("p h w -> p (h w)")
            xbf = xpadB[:].rearrange("p h w -> p (h w)")
            nc.vector.tensor_copy(xbf[:, 0 : HP * WP - 1], xpf[:, 1 : HP * WP])
            wb2 = wbias[:]
            wn2 = wneg[:]

            psums = [
                ppool.tile([128, CHUNK], f32, name=f"ps{f}", tag=f"ps{f}")
                for f in range(NCHUNK)
            ]

            taps = [(kh, kw) for kh in range(K) for kw in range(K)]
            # act-tap distribution (total 36 tap-slots = 144 act tiles):
            # cc0 gets a single act tap (8,) so the first PE block is not
            # gated on the ScalarE warm-up; ACT3 blocks get three.
            ACT3 = (2, 4, 7, 9, 11, 14)
            def act_taps(cc):
                if cc == 0:
                    return (8,)
                return (6, 7, 8) if cc in ACT3 else (7, 8)

            def tap_src(t):
                """[128, 56, 56] window of the padded plane for tap t,
                4B-aligned for the DVE (odd kw reads the shifted copy)."""
                kh, kw = taps[t]
                if kw == 1:
                    return xpadB[:, kh : kh + H, 0:W]
                return xpad[:, kh : kh + H, kw : kw + W]


            # start-flag bookkeeping: first MM to touch each (g, chunk)
            # PSUM region must carry start=True (emission order == PE order)
            started = [[False] * NCHUNK for _ in range(4)]

            def mm(f, g, lhsT, rhs, stop=False):
                st = not started[g][f]
                started[g][f] = True
                nc.tensor.matmul(
                    psums[f][32 * g : 32 * g + 32, :],
                    lhsT,
                    rhs,
                    start=st,
                    stop=stop,
                    tile_position=(0, 32 * g),
                )

            # sum-x matmul groups, spread through the cc blocks below
            sumx_groups = [(t, f) for t in range(8) for f in range(NCHUNK)]

            def emit_sumx(t, f):
                kh, kw = taps[t]
                r0 = f * CHUNK_ROWS
                rhs = xpad[:, r0 + kh : r0 + kh + CHUNK_ROWS, kw : kw + W]
                lhsT = {6: statx6[:], 7: statx7[:]}.get(t, statn[:])
                for g in range(4):
                    mm(f, g, lhsT, rhs)

            # ---- main loop -------------------------------------------
            # DVE tiles: y = min(x, w)   (contiguous output) -> stationary +2
            # ACT tiles: y = |x - w|                         -> stationary -1
            # ACT taps emitted first within each cc block: ScalarE runs one
            # cc ahead (separate pool), so its tiles are ready when the PE
            # stream reaches the act matmuls.
            for cc in range(16):
                lo = (len(sumx_groups) * cc) // 16
                hi = (len(sumx_groups) * (cc + 1)) // 16
                for t, f in sumx_groups[lo:hi]:
                    emit_sumx(t, f)
                at = act_taps(cc)
                if cc == 0:
                    # first block: ACT starts ~10us late (DMA+cast deps);
                    # consume its tiles at the END of the block instead.
                    tap_order = [t for t in range(9) if t not in at] + list(at)
                else:
                    tap_order = list(at) + [t for t in range(9) if t not in at]
                for t in tap_order:
                    is_act = t in at
                    for g in range(4):
                        co = 16 * g + cc
                        if not is_act:
                            y = ypool.tile([128, L], bf16, tag="y")
                            nc.vector.tensor_scalar(
                                y[:].rearrange("p (a b) -> p a b", b=W),
                                tap_src(t),
                                wb2[:, co * 9 + t : co * 9 + t + 1],
                                None,
                                op0=mybir.AluOpType.min,
                            )
                            lhsT = stat2[:, cc, :]
                        else:
                            y = apool.tile([128, L], bf16, tag="ya")
                            nc.scalar.activation(
                                y[:].rearrange("p (a b) -> p a b", b=W),
                                tap_src(t),
                                mybir.ActivationFunctionType.Abs,
                                bias=wn2[:, co * 9 + t : co * 9 + t + 1],
                            )
                            lhsT = statm[:, cc, :]
                        for f in range(NCHUNK):
                            rhs = y[:, f * CHUNK : (f + 1) * CHUNK]
                            mm(
                                f, g, lhsT, rhs,
                                stop=(cc == 15 and t == tap_order[-1]),
                            )

            # ---- epilogue: out = psum + (-sum w), psum -> sbuf -> dram --
            for f in range(NCHUNK):
                nc.scalar.activation(
                    out_sb[:, f * CHUNK : (f + 1) * CHUNK],
                    psums[f][:],
                    mybir.ActivationFunctionType.Identity,
                    bias=swn[:],
                )
            out_engs = [nc.sync, nc.gpsimd, nc.scalar, nc.sync]
            for q in range(4):
                out_engs[q].dma_start(
                    o_d.ap()[32 * q : 32 * (q + 1), :],
                    out_sb[32 * q : 32 * (q + 1), :],
                )

    nc.compile()
    return nc


def get_nc():
    global _nc_cache
    if _nc_cache is None:
        _nc_cache = build_nc()
    return _nc_cache


def make_in_maps(x, w):
    import ml_dtypes

    x = np.ascontiguousarray(x).astype(ml_dtypes.bfloat16)
    w = np.ascontiguousarray(w, dtype=np.float32)
    wr = w.reshape(CO, C, K * K)
    # wb[p = img*64 + ci, co*9 + t] = w[co, ci, t]  (both img halves)
    wb_half = wr.transpose(1, 0, 2).reshape(C, CO * K * K)
    wb = np.ascontiguousarray(np.vstack([wb_half, wb_half]), dtype=np.float32)
    wn = np.ascontiguousarray(-wb)
    # -sum w[co] over each co's min-trick taps: t 0..5 always; t6 when
    # cc=co%16 not in ACT3={2,5,8,11,14}; t7 only when cc==0; t8 never.
    # Scattered to psum partitions p = 32*(co//16)+2*(co%16)+img.
    swc = -wr[:, :, :6].sum(axis=(1, 2))
    mask6 = np.array([(co % 16) not in (2, 4, 7, 9, 11, 14) for co in range(CO)])
    swc -= np.where(mask6, wr[:, :, 6].sum(axis=1), 0.0)
    mask7 = np.array([(co % 16) == 0 for co in range(CO)])
    swc -= np.where(mask7, wr[:, :, 7].sum(axis=1), 0.0)
    swn = np.empty((128, 1), dtype=np.float32)
    for co in range(CO):
        p = 32 * (co // 16) + 2 * (co % 16)
        swn[p, 0] = swc[co]
        swn[p + 1, 0] = swc[co]
    return [
        {"x": x[i * NLOC : (i + 1) * NLOC], "wb": wb, "wn": wn, "swn": swn}
        for i in range(NCORES)
    ]


def unscramble(core_out):
    """[128, L] with row p = 32*(co//16) + 2*(co%16) + img -> [2, 64, 56, 56]."""
    return (
        core_out.reshape(4, 16, NLOC, H, W)
        .transpose(2, 0, 1, 3, 4)
        .reshape(NLOC, CO, H, W)
    )


def kernel(x, w):
    nc = get_nc()
    res = run_bass_kernel_spmd(nc, make_in_maps(x, w), core_ids=list(range(NCORES)))
    out = np.concatenate([unscramble(r["out"]) for r in res.results], axis=0)
    return np.ascontiguousarray(out, dtype=np.float32)


if __name__ == "__main__":
    x = np.random.randn(N, C, H, W).astype(np.float32)
    w = np.random.randn(CO, C, K, K).astype(np.float32)
    o = kernel(x, w)
    print("out", o.shape, o.dtype, float(o.mean()))


# revision 48
# speedup vs baseline: 2.1058x; 1.0692x over previous
"""AdderNet 2D convolution on 8 TRN2 NeuronCores.

out[n,co,h,w] = -sum_{ci,kh,kw} |xpad[n,ci,h+kh,w+kw] - w[co,ci,kh,kw]|

Sharding: data-parallel over the batch dim (16 images -> 2 per core),
weight replicated.  No collectives needed (forward pass only).

Math: |x - w| = x + w - 2*min(x, w), so

  -sum |x - w| = 2*sum min(x, w) - sum x - sum w

The heavy term is one single-op DVE tensor_scalar per (co, tap):
y = min(xpad, w[co,ci,kh,kw]) in bf16 (4x perf mode), evaluated over the
full padded plane so every instruction is contiguous/aligned.  TensorE
reduces partitions with a constant +2 block stationary into PSUM
(accumulating the 9 taps); the (kh,kw) tap shift is applied by the
matmul's strided moving-view.  "sum x" is accumulated by 252 extra
matmuls with an all-(-1) stationary; "sum w" comes in as a tiny
host-precomputed per-partition bias, applied in the epilogue.

Per-core layout:
  - 128 SBUF partitions = img*64 + ci  (2 images per core)
  - psum/output partition p = 32*(co//16) + 2*(co%16) + img
    (TensorE column-tiling: 4 strips of 32, one per co-group)
  - zero padding in xpad contributes min(0, w) terms and the matching
    zeros in sum x, exactly reproducing the reference's |0 - w| border
    terms.
"""

import numpy as np

try:
    from concourse import bacc, mybir, tile
except ImportError:  # pragma: no cover - fallback when sitecustomize absent
    import sys

    sys.path.insert(0, "/opt/trn_rl_repo")
    from concourse import bacc, mybir, tile

from concourse.bass_utils import run_bass_kernel_spmd

N, C, H, W = 16, 64, 56, 56
CO, K = 64, 3
NCORES = 8
NLOC = N // NCORES  # images per core = 2
HP = H + 2  # padded plane height
WP = W + 2
L = H * W  # 3136 output pixels
CHUNK_ROWS = 8  # output rows per psum bank chunk
NCHUNK = H // CHUNK_ROWS  # 7
CHUNK = CHUNK_ROWS * W  # 448 <= 512 fp32 / psum bank

_nc_cache = None


def build_nc():
    nc = bacc.Bacc(
        "TRN2",
        target_bir_lowering=False,
        debug=False,
        num_devices=NCORES,
    )
    f32 = mybir.dt.float32
    bf16 = mybir.dt.bfloat16

    # host-padded bf16 plane: x[p = img*64 + ci, (h, w) of the 58x58
    # zero-bordered image]
    x_d = nc.dram_tensor("x", [128, HP * WP], bf16, kind="ExternalInput")
    # host-preshuffled weights: wb[p=img*64+ci, co*9+t] = w[co, ci, t]
    wb_d = nc.dram_tensor("wb", [128, CO * K * K], f32, kind="ExternalInput")
    wn_d = nc.dram_tensor("wn", [128, CO * K * K], f32, kind="ExternalInput")
    # swn[p, 0] = -sum_{ci, min-taps} w[co(p)] at psum partition p (host side)
    swn_d = nc.dram_tensor("swn", [128, 1], f32, kind="ExternalInput")
    # out rows are psum-partition-major: p = 32*(co//16) + 2*(co%16) + img;
    # the host-side gather untangles this ordering (cheap numpy transpose).
    o_d = nc.dram_tensor("out", [128, L], f32, kind="ExternalOutput")

    with tile.TileContext(nc) as tc:
        with (
            tc.tile_pool(name="const", bufs=1) as cpool,
            tc.tile_pool(name="ypool", bufs=12) as ypool,
            tc.tile_pool(name="apool", bufs=16) as apool,
            tc.tile_pool(name="psum", bufs=1, space="PSUM") as ppool,
        ):
            xpad = cpool.tile([128, HP, WP], bf16)
            # xpadB[p, r, c] = xpad[p, r, c+1]: left-shifted copy so the
            # kw==1 DVE taps read from a 4-byte-aligned window (keeps the
            # tensor_scalar in 4x perf mode).
            xpadB = cpool.tile([128, HP, WP], bf16)
            wbias = cpool.tile([128, CO * K * K], f32)
            wneg = cpool.tile([128, CO * K * K], f32)
            swn = cpool.tile([128, 1], f32)
            # stat2[:, c, :]: [128, 32] stationary, col 2c+i = +2 on the
            # img-i partition half, else 0  (the 2*min reduction).
            stat2 = cpool.tile([128, 16, 32], bf16)
            # statm[:, c, :]: same pattern with -1 (the -|x-w| ACT tiles).
            statm = cpool.tile([128, 16, 32], bf16)
            # statn: [128, 32] all-columns -1 on matching img half (sum-x).
            statn = cpool.tile([128, 32], bf16)
            # statx6/statx7: like statn but only over the co columns for
            # which that tap runs on the DVE (min-trick) path.
            statx6 = cpool.tile([128, 32], bf16)
            statx7 = cpool.tile([128, 32], bf16)
            out_sb = cpool.tile([128, L], f32)

            # ---- loads -------------------------------------------------
            # contiguous DMA of the host-padded plane, split across rings
            xv = x_d.ap()
            xpflat = xpad[:].rearrange("p h w -> p (h w)")
            dma_engs = [nc.sync, nc.gpsimd, nc.scalar, nc.sync]
            for q in range(4):
                dma_engs[q].dma_start(
                    xpflat[32 * q : 32 * (q + 1), :],
                    xv[32 * q : 32 * (q + 1), :],
                )
            nc.gpsimd.dma_start(wbias[:], wb_d.ap())
            nc.gpsimd.dma_start(wneg[:], wn_d.ap())
            nc.sync.dma_start(swn[:], swn_d.ap())

            # ---- constants --------------------------------------------
            nc.vector.memset(stat2[:], 0.0)
            nc.vector.memset(statm[:], 0.0)
            for c in range(16):
                nc.vector.memset(stat2[0:64, c, 2 * c : 2 * c + 1], 2.0)
                nc.vector.memset(stat2[64:128, c, 2 * c + 1 : 2 * c + 2], 2.0)
                nc.vector.memset(statm[0:64, c, 2 * c : 2 * c + 1], -1.0)
                nc.vector.memset(statm[64:128, c, 2 * c + 1 : 2 * c + 2], -1.0)
            nc.vector.memset(statn[:], 0.0)
            nc.vector.memset(statn[0:64, 0:32:2], -1.0)
            nc.vector.memset(statn[64:128, 1:32:2], -1.0)
            nc.vector.memset(statx6[:], 0.0)
            for c in range(16):
                if c not in (2, 4, 7, 9, 11, 14):
                    nc.vector.memset(statx6[0:64, 2 * c : 2 * c + 1], -1.0)
                    nc.vector.memset(statx6[64:128, 2 * c + 1 : 2 * c + 2], -1.0)
            nc.vector.memset(statx7[:], 0.0)
            nc.vector.memset(statx7[0:64, 0:1], -1.0)
            nc.vector.memset(statx7[64:128, 1:2], -1.0)

            # ---- shifted copy for the odd-kw taps ---------------------
            xpf = xpad[:].rearrange("p h w -> p (h w)")
            xbf = xpadB[:].rearrange("p h w -> p (h w)")
            nc.vector.tensor_copy(xbf[:, 0 : HP * WP - 1], xpf[:, 1 : HP * WP])
            wb2 = wbias[:]
            wn2 = wneg[:]

            psums = [
                ppool.tile([128, CHUNK], f32, name=f"ps{f}", tag=f"ps{f}")
                for f in range(NCHUNK)
            ]

            taps = [(kh, kw) for kh in range(K) for kw in range(K)]
            # act-tap distribution (total 36 tap-slots = 144 act tiles):
            # cc0 gets a single act tap (8,) so the first PE block is not
            # gated on the ScalarE warm-up; ACT3 blocks get three.
            ACT3 = (2, 4, 7, 9, 11, 14)
            def act_taps(cc):
                if cc == 0:
                    return (8,)
                return (6, 7, 8) if cc in ACT3 else (7, 8)

            def tap_src(t):
                """[128, 56, 56] window of the padded plane for tap t,
                4B-aligned for the DVE (odd kw reads the shifted copy)."""
                kh, kw = taps[t]
                if kw == 1:
                    return xpadB[:, kh : kh + H, 0:W]
                return xpad[:, kh : kh + H, kw : kw + W]


            # start-flag bookkeeping: first MM to touch each (g, chunk)
            # PSUM region must carry start=True (emission order == PE order)
            started = [[False] * NCHUNK for _ in range(4)]

            def mm(f, g, lhsT, rhs, stop=False):
                st = not started[g][f]
                started[g][f] = True
                nc.tensor.matmul(
                    psums[f][32 * g : 32 * g + 32, :],
                    lhsT,
                    rhs,
                    start=st,
                    stop=stop,
                    tile_position=(0, 32 * g),
                )

            # sum-x matmul groups, spread through the cc blocks below
            sumx_groups = [(t, f) for t in range(8) for f in range(NCHUNK)]

            def emit_sumx(t, f):
                kh, kw = taps[t]
                r0 = f * CHUNK_ROWS
                rhs = xpad[:, r0 + kh : r0 + kh + CHUNK_ROWS, kw : kw + W]
                lhsT = {6: statx6[:], 7: statx7[:]}.get(t, statn[:])
                for g in range(4):
                    mm(f, g, lhsT, rhs)

            # ---- main loop -------------------------------------------
            # DVE tiles: y = min(x, w)   (contiguous output) -> stationary +2
            # ACT tiles: y = |x - w|                         -> stationary -1
            # ACT taps emitted first within each cc block: ScalarE runs one
            # cc ahead (separate pool), so its tiles are ready when the PE
            # stream reaches the act matmuls.
            for cc in range(16):
                lo = (len(sumx_groups) * cc) // 16
                hi = (len(sumx_groups) * (cc + 1)) // 16
                for t, f in sumx_groups[lo:hi]:
                    emit_sumx(t, f)
                at = act_taps(cc)
                if cc == 0:
                    # first block: ACT starts ~10us late (DMA+cast deps);
                    # consume its tiles at the END of the block instead.
                    tap_order = [t for t in range(9) if t not in at] + list(at)
                else:
                    tap_order = list(at) + [t for t in range(9) if t not in at]
                for t in tap_order:
                    is_act = t in at
                    for g in range(4):
                        co = 16 * g + cc
                        if not is_act:
                            y = ypool.tile([128, L], bf16, tag="y")
                            nc.vector.tensor_scalar(
                                y[:].rearrange("p (a b) -> p a b", b=W),
                                tap_src(t),
                                wb2[:, co * 9 + t : co * 9 + t + 1],
                                None,
                                op0=mybir.AluOpType.min,
                            )
                            lhsT = stat2[:, cc, :]
                        else:
                            y = apool.tile([128, L], bf16, tag="ya")
                            nc.scalar.activation(
                                y[:].rearrange("p (a b) -> p a b", b=W),
                                tap_src(t),
                                mybir.ActivationFunctionType.Abs,
                                bias=wn2[:, co * 9 + t : co * 9 + t + 1],
                            )
                            lhsT = statm[:, cc, :]
                        for f in range(NCHUNK):
                            rhs = y[:, f * CHUNK : (f + 1) * CHUNK]
                            mm(
                                f, g, lhsT, rhs,
                                stop=(cc == 15 and t == tap_order[-1]),
                            )

            # ---- epilogue: out = psum + (-sum w), psum -> sbuf -> dram --
            for f in range(NCHUNK):
                nc.scalar.activation(
                    out_sb[:, f * CHUNK : (f + 1) * CHUNK],
                    psums[f][:],
                    mybir.ActivationFunctionType.Identity,
                    bias=swn[:],
                )
            out_engs = [nc.sync, nc.gpsimd, nc.scalar, nc.sync]
            for q in range(4):
                out_engs[q].dma_start(
                    o_d.ap()[32 * q : 32 * (q + 1), :],
                    out_sb[32 * q : 32 * (q + 1), :],
                )

    nc.compile()
    return nc


def get_nc():
    global _nc_cache
    if _nc_cache is None:
        _nc_cache = build_nc()
    return _nc_cache


def make_in_maps(x, w):
    import ml_dtypes

    xb = np.ascontiguousarray(x).astype(ml_dtypes.bfloat16)
    xpad_all = np.zeros((N * C, HP, WP), dtype=ml_dtypes.bfloat16)
    xpad_all[:, 1 : H + 1, 1 : W + 1] = xb.reshape(N * C, H, W)
    xpad_all = xpad_all.reshape(N, C, HP * WP)
    w = np.ascontiguousarray(w, dtype=np.float32)
    wr = w.reshape(CO, C, K * K)
    # wb[p = img*64 + ci, co*9 + t] = w[co, ci, t]  (both img halves)
    wb_half = wr.transpose(1, 0, 2).reshape(C, CO * K * K)
    wb = np.ascontiguousarray(np.vstack([wb_half, wb_half]), dtype=np.float32)
    wn = np.ascontiguousarray(-wb)
    # -sum w[co] over each co's min-trick taps: t 0..5 always; t6 when
    # cc=co%16 not in ACT3={2,5,8,11,14}; t7 only when cc==0; t8 never.
    # Scattered to psum partitions p = 32*(co//16)+2*(co%16)+img.
    swc = -wr[:, :, :6].sum(axis=(1, 2))
    mask6 = np.array([(co % 16) not in (2, 4, 7, 9, 11, 14) for co in range(CO)])
    swc -= np.where(mask6, wr[:, :, 6].sum(axis=1), 0.0)
    mask7 = np.array([(co % 16) == 0 for co in range(CO)])
    swc -= np.where(mask7, wr[:, :, 7].sum(axis=1), 0.0)
    swn = np.empty((128, 1), dtype=np.float32)
    for co in range(CO):
        p = 32 * (co // 16) + 2 * (co % 16)
        swn[p, 0] = swc[co]
        swn[p + 1, 0] = swc[co]
    return [
        {
            "x": np.ascontiguousarray(
                xpad_all[i * NLOC : (i + 1) * NLOC].reshape(128, HP * WP)
            ),
            "wb": wb,
            "wn": wn,
            "swn": swn,
        }
        for i in range(NCORES)
    ]


def unscramble(core_out):
    """[128, L] with row p = 32*(co//16) + 2*(co%16) + img -> [2, 64, 56, 56]."""
    return (
        core_out.reshape(4, 16, NLOC, H, W)
        .transpose(2, 0, 1, 3, 4)
        .reshape(NLOC, CO, H, W)
    )


def kernel(x, w):
    nc = get_nc()
    res = run_bass_kernel_spmd(nc, make_in_maps(x, w), core_ids=list(range(NCORES)))
    out = np.concatenate([unscramble(r["out"]) for r in res.results], axis=0)
    return np.ascontiguousarray(out, dtype=np.float32)


if __name__ == "__main__":
    x = np.random.randn(N, C, H, W).astype(np.float32)
    w = np.random.randn(CO, C, K, K).astype(np.float32)
    o = kernel(x, w)
    print("out", o.shape, o.dtype, float(o.mean()))


# revision 49
# speedup vs baseline: 2.1300x; 1.0115x over previous
"""AdderNet 2D convolution on 8 TRN2 NeuronCores.

out[n,co,h,w] = -sum_{ci,kh,kw} |xpad[n,ci,h+kh,w+kw] - w[co,ci,kh,kw]|

Sharding: data-parallel over the batch dim (16 images -> 2 per core),
weight replicated.  No collectives needed (forward pass only).

Math: |x - w| = x + w - 2*min(x, w), so

  -sum |x - w| = 2*sum min(x, w) - sum x - sum w

The heavy term is one single-op DVE tensor_scalar per (co, tap):
y = min(xpad, w[co,ci,kh,kw]) in bf16 (4x perf mode), evaluated over the
full padded plane so every instruction is contiguous/aligned.  TensorE
reduces partitions with a constant +2 block stationary into PSUM
(accumulating the 9 taps); the (kh,kw) tap shift is applied by the
matmul's strided moving-view.  "sum x" is accumulated by 252 extra
matmuls with an all-(-1) stationary; "sum w" comes in as a tiny
host-precomputed per-partition bias, applied in the epilogue.

Per-core layout:
  - 128 SBUF partitions = img*64 + ci  (2 images per core)
  - psum/output partition p = 32*(co//16) + 2*(co%16) + img
    (TensorE column-tiling: 4 strips of 32, one per co-group)
  - zero padding in xpad contributes min(0, w) terms and the matching
    zeros in sum x, exactly reproducing the reference's |0 - w| border
    terms.
"""

import numpy as np

try:
    from concourse import bacc, mybir, tile
except ImportError:  # pragma: no cover - fallback when sitecustomize absent
    import sys

    sys.path.insert(0, "/opt/trn_rl_repo")
    from concourse import bacc, mybir, tile

from concourse.bass_utils import run_bass_kernel_spmd

N, C, H, W = 16, 64, 56, 56
CO, K = 64, 3
NCORES = 8
NLOC = N // NCORES  # images per core = 2
HP = H + 2  # padded plane height
WP = W + 2
L = H * W  # 3136 output pixels
CHUNK_ROWS = 8  # output rows per psum bank chunk
NCHUNK = H // CHUNK_ROWS  # 7
CHUNK = CHUNK_ROWS * W  # 448 <= 512 fp32 / psum bank

_nc_cache = None


def build_nc():
    nc = bacc.Bacc(
        "TRN2",
        target_bir_lowering=False,
        debug=False,
        num_devices=NCORES,
    )
    f32 = mybir.dt.float32
    bf16 = mybir.dt.bfloat16

    # host-padded bf16 plane: x[p = img*64 + ci, (h, w) of the 58x58
    # zero-bordered image]
    x_d = nc.dram_tensor("x", [128, HP * WP], bf16, kind="ExternalInput")
    xs_d = nc.dram_tensor("xs", [128, HP * WP], bf16, kind="ExternalInput")
    # host-preshuffled weights: wb[p=img*64+ci, co*9+t] = w[co, ci, t]
    wb_d = nc.dram_tensor("wb", [128, CO * K * K], f32, kind="ExternalInput")
    wn_d = nc.dram_tensor("wn", [128, CO * K * K], f32, kind="ExternalInput")
    # swn[p, 0] = -sum_{ci, min-taps} w[co(p)] at psum partition p (host side)
    swn_d = nc.dram_tensor("swn", [128, 1], f32, kind="ExternalInput")
    # out rows are psum-partition-major: p = 32*(co//16) + 2*(co%16) + img;
    # the host-side gather untangles this ordering (cheap numpy transpose).
    o_d = nc.dram_tensor("out", [128, L], f32, kind="ExternalOutput")

    with tile.TileContext(nc) as tc:
        with (
            tc.tile_pool(name="const", bufs=1) as cpool,
            tc.tile_pool(name="ypool", bufs=12) as ypool,
            tc.tile_pool(name="apool", bufs=16) as apool,
            tc.tile_pool(name="psum", bufs=1, space="PSUM") as ppool,
        ):
            xpad = cpool.tile([128, HP, WP], bf16)
            # xpadB[p, r, c] = xpad[p, r, c+1]: left-shifted copy so the
            # kw==1 DVE taps read from a 4-byte-aligned window (keeps the
            # tensor_scalar in 4x perf mode).
            xpadB = cpool.tile([128, HP, WP], bf16)
            wbias = cpool.tile([128, CO * K * K], f32)
            wneg = cpool.tile([128, CO * K * K], f32)
            swn = cpool.tile([128, 1], f32)
            # stat2[:, c, :]: [128, 32] stationary, col 2c+i = +2 on the
            # img-i partition half, else 0  (the 2*min reduction).
            stat2 = cpool.tile([128, 16, 32], bf16)
            # statm[:, c, :]: same pattern with -1 (the -|x-w| ACT tiles).
            statm = cpool.tile([128, 16, 32], bf16)
            # statn: [128, 32] all-columns -1 on matching img half (sum-x).
            statn = cpool.tile([128, 32], bf16)
            # statx6/statx7: like statn but only over the co columns for
            # which that tap runs on the DVE (min-trick) path.
            statx6 = cpool.tile([128, 32], bf16)
            statx7 = cpool.tile([128, 32], bf16)
            out_sb = cpool.tile([128, L], f32)

            # ---- loads -------------------------------------------------
            # contiguous DMA of the host-padded planes, split across rings
            xpflat = xpad[:].rearrange("p h w -> p (h w)")
            xsflat = xpadB[:].rearrange("p h w -> p (h w)")
            dma_engs = [nc.sync, nc.gpsimd, nc.scalar, nc.sync,
                        nc.gpsimd, nc.scalar, nc.sync, nc.gpsimd]
            for q in range(4):
                dma_engs[q].dma_start(
                    xpflat[32 * q : 32 * (q + 1), :],
                    x_d.ap()[32 * q : 32 * (q + 1), :],
                )
                dma_engs[q + 4].dma_start(
                    xsflat[32 * q : 32 * (q + 1), :],
                    xs_d.ap()[32 * q : 32 * (q + 1), :],
                )
            nc.gpsimd.dma_start(wbias[:], wb_d.ap())
            nc.gpsimd.dma_start(wneg[:], wn_d.ap())
            nc.sync.dma_start(swn[:], swn_d.ap())

            # ---- constants --------------------------------------------
            nc.vector.memset(stat2[:], 0.0)
            nc.vector.memset(statm[:], 0.0)
            for c in range(16):
                nc.vector.memset(stat2[0:64, c, 2 * c : 2 * c + 1], 2.0)
                nc.vector.memset(stat2[64:128, c, 2 * c + 1 : 2 * c + 2], 2.0)
                nc.vector.memset(statm[0:64, c, 2 * c : 2 * c + 1], -1.0)
                nc.vector.memset(statm[64:128, c, 2 * c + 1 : 2 * c + 2], -1.0)
            nc.vector.memset(statn[:], 0.0)
            nc.vector.memset(statn[0:64, 0:32:2], -1.0)
            nc.vector.memset(statn[64:128, 1:32:2], -1.0)
            nc.vector.memset(statx6[:], 0.0)
            for c in range(16):
                if c not in (2, 4, 7, 9, 11, 14):
                    nc.vector.memset(statx6[0:64, 2 * c : 2 * c + 1], -1.0)
                    nc.vector.memset(statx6[64:128, 2 * c + 1 : 2 * c + 2], -1.0)
            nc.vector.memset(statx7[:], 0.0)
            nc.vector.memset(statx7[0:64, 0:1], -1.0)
            nc.vector.memset(statx7[64:128, 1:2], -1.0)

            wb2 = wbias[:]
            wn2 = wneg[:]

            psums = [
                ppool.tile([128, CHUNK], f32, name=f"ps{f}", tag=f"ps{f}")
                for f in range(NCHUNK)
            ]

            taps = [(kh, kw) for kh in range(K) for kw in range(K)]
            # act-tap distribution (total 36 tap-slots = 144 act tiles):
            # cc0 gets a single act tap (8,) so the first PE block is not
            # gated on the ScalarE warm-up; ACT3 blocks get three.
            ACT3 = (2, 4, 7, 9, 11, 14)
            def act_taps(cc):
                if cc == 0:
                    return (8,)
                return (6, 7, 8) if cc in ACT3 else (7, 8)

            def tap_src(t):
                """[128, 56, 56] window of the padded plane for tap t,
                4B-aligned for the DVE (odd kw reads the shifted copy)."""
                kh, kw = taps[t]
                if kw == 1:
                    return xpadB[:, kh : kh + H, 0:W]
                return xpad[:, kh : kh + H, kw : kw + W]


            # start-flag bookkeeping: first MM to touch each (g, chunk)
            # PSUM region must carry start=True (emission order == PE order)
            started = [[False] * NCHUNK for _ in range(4)]

            def mm(f, g, lhsT, rhs, stop=False):
                st = not started[g][f]
                started[g][f] = True
                nc.tensor.matmul(
                    psums[f][32 * g : 32 * g + 32, :],
                    lhsT,
                    rhs,
                    start=st,
                    stop=stop,
                    tile_position=(0, 32 * g),
                )

            # sum-x matmul groups, spread through the cc blocks below
            sumx_groups = [(t, f) for t in range(8) for f in range(NCHUNK)]

            def emit_sumx(t, f):
                kh, kw = taps[t]
                r0 = f * CHUNK_ROWS
                rhs = xpad[:, r0 + kh : r0 + kh + CHUNK_ROWS, kw : kw + W]
                lhsT = {6: statx6[:], 7: statx7[:]}.get(t, statn[:])
                for g in range(4):
                    mm(f, g, lhsT, rhs)

            # ---- main loop -------------------------------------------
            # DVE tiles: y = min(x, w)   (contiguous output) -> stationary +2
            # ACT tiles: y = |x - w|                         -> stationary -1
            # ACT taps emitted first within each cc block: ScalarE runs one
            # cc ahead (separate pool), so its tiles are ready when the PE
            # stream reaches the act matmuls.
            for cc in range(16):
                lo = (len(sumx_groups) * cc) // 16
                hi = (len(sumx_groups) * (cc + 1)) // 16
                for t, f in sumx_groups[lo:hi]:
                    emit_sumx(t, f)
                at = act_taps(cc)
                mins = [t for t in range(9) if t not in at]
                if cc == 0:
                    # first block: ACT ramps up behind the DMA; consume its
                    # tile at the END of the block.
                    tap_order = mins + list(at)
                else:
                    # interleave act-consuming groups between min groups so
                    # the PE never needs a long run of banked ACT tiles
                    tap_order = []
                    step = len(mins) // len(at) + 1
                    ai, mi = 0, 0
                    for k in range(9):
                        if k % step == 1 and ai < len(at):
                            tap_order.append(at[ai]); ai += 1
                        elif mi < len(mins):
                            tap_order.append(mins[mi]); mi += 1
                        elif ai < len(at):
                            tap_order.append(at[ai]); ai += 1
                for t in tap_order:
                    is_act = t in at
                    for g in range(4):
                        co = 16 * g + cc
                        if not is_act:
                            y = ypool.tile([128, L], bf16, tag="y")
                            nc.vector.tensor_scalar(
                                y[:].rearrange("p (a b) -> p a b", b=W),
                                tap_src(t),
                                wb2[:, co * 9 + t : co * 9 + t + 1],
                                None,
                                op0=mybir.AluOpType.min,
                            )
                            lhsT = stat2[:, cc, :]
                        else:
                            y = apool.tile([128, L], bf16, tag="ya")
                            nc.scalar.activation(
                                y[:].rearrange("p (a b) -> p a b", b=W),
                                tap_src(t),
                                mybir.ActivationFunctionType.Abs,
                                bias=wn2[:, co * 9 + t : co * 9 + t + 1],
                            )
                            lhsT = statm[:, cc, :]
                        for f in range(NCHUNK):
                            rhs = y[:, f * CHUNK : (f + 1) * CHUNK]
                            mm(
                                f, g, lhsT, rhs,
                                stop=(cc == 15 and t == tap_order[-1]),
                            )

            # ---- epilogue: out = psum + (-sum w), psum -> sbuf -> dram --
            for f in range(NCHUNK):
                nc.vector.tensor_scalar(
                    out_sb[:, f * CHUNK : (f + 1) * CHUNK],
                    psums[f][:],
                    swn[:],
                    None,
                    op0=mybir.AluOpType.add,
                )
            out_engs = [nc.sync, nc.gpsimd, nc.scalar, nc.sync]
            for q in range(4):
                out_engs[q].dma_start(
                    o_d.ap()[32 * q : 32 * (q + 1), :],
                    out_sb[32 * q : 32 * (q + 1), :],
                )

    nc.compile()
    return nc


def get_nc():
    global _nc_cache
    if _nc_cache is None:
        _nc_cache = build_nc()
    return _nc_cache


def make_in_maps(x, w):
    import ml_dtypes

    xb = np.ascontiguousarray(x).astype(ml_dtypes.bfloat16)
    xpad_all = np.zeros((N * C, HP, WP), dtype=ml_dtypes.bfloat16)
    xpad_all[:, 1 : H + 1, 1 : W + 1] = xb.reshape(N * C, H, W)
    xpad_all = xpad_all.reshape(N, C, HP * WP)
    xs_all = np.zeros_like(xpad_all)
    xs_all[:, :, 0 : HP * WP - 1] = xpad_all[:, :, 1:]
    w = np.ascontiguousarray(w, dtype=np.float32)
    wr = w.reshape(CO, C, K * K)
    # wb[p = img*64 + ci, co*9 + t] = w[co, ci, t]  (both img halves)
    wb_half = wr.transpose(1, 0, 2).reshape(C, CO * K * K)
    wb = np.ascontiguousarray(np.vstack([wb_half, wb_half]), dtype=np.float32)
    wn = np.ascontiguousarray(-wb)
    # -sum w[co] over each co's min-trick taps: t 0..5 always; t6 when
    # cc=co%16 not in ACT3={2,5,8,11,14}; t7 only when cc==0; t8 never.
    # Scattered to psum partitions p = 32*(co//16)+2*(co%16)+img.
    swc = -wr[:, :, :6].sum(axis=(1, 2))
    mask6 = np.array([(co % 16) not in (2, 4, 7, 9, 11, 14) for co in range(CO)])
    swc -= np.where(mask6, wr[:, :, 6].sum(axis=1), 0.0)
    mask7 = np.array([(co % 16) == 0 for co in range(CO)])
    swc -= np.where(mask7, wr[:, :, 7].sum(axis=1), 0.0)
    swn = np.empty((128, 1), dtype=np.float32)
    for co in range(CO):
        p = 32 * (co // 16) + 2 * (co % 16)
        swn[p, 0] = swc[co]
        swn[p + 1, 0] = swc[co]
    return [
        {
            "x": np.ascontiguousarray(
                xpad_all[i * NLOC : (i + 1) * NLOC].reshape(128, HP * WP)
            ),
            "xs": np.ascontiguousarray(
                xs_all[i * NLOC : (i + 1) * NLOC].reshape(128, HP * WP)
            ),
            "wb": wb,
            "wn": wn,
            "swn": swn,
        }
        for i in range(NCORES)
    ]


def unscramble(core_out):
    """[128, L] with row p = 32*(co//16) + 2*(co%16) + img -> [2, 64, 56, 56]."""
    return (
        core_out.reshape(4, 16, NLOC, H, W)
        .transpose(2, 0, 1, 3, 4)
        .reshape(NLOC, CO, H, W)
    )


def kernel(x, w):
    nc = get_nc()
    res = run_bass_kernel_spmd(nc, make_in_maps(x, w), core_ids=list(range(NCORES)))
    out = np.concatenate([unscramble(r["out"]) for r in res.results], axis=0)
    return np.ascontiguousarray(out, dtype=np.float32)


if __name__ == "__main__":
    x = np.random.randn(N, C, H, W).astype(np.float32)
    w = np.random.randn(CO, C, K, K).astype(np.float32)
    o = kernel(x, w)
    print("out", o.shape, o.dtype, float(o.mean()))
